# revision 11
# baseline (speedup 1.0000x reference)
"""Trainium2 Bass kernel for nn_Block2DGRU (norm->dwconv3x3->bi-minGRU->norm->MLP).

Self-contained: host-side weight folding + sharding, device kernel via
Bass/Tile, SPMD over 8 NeuronCores (data-parallel over batch: 2 per core).

Device layout: everything [feature_on_partitions, time_on_free].  The minGRU
linear recurrence h_t = a_t*h_{t-1} + b_t runs natively on the DVE via
tensor_tensor_scan (per-partition scan along the free dim); GRU2 is the same
scan with reversed access patterns.

Precision: weights and activations are bf16 (converted on host; 1 cyc/row on
the PE like f32r, but halves SBUF/DMA and doubles 16-bit DVE throughput).
The y accumulator (residual + norm2 input) stays f32r; the x residual is
re-read from an f32 copy in DRAM.  PSUM accumulation is always fp32.
"""
import numpy as np
import ml_dtypes

import concourse.bass as bass
import concourse.tile as tile
import concourse.mybir as mybir
from concourse.bass_utils import run_bass_kernel_spmd

F32 = mybir.dt.float32
F32R = mybir.dt.float32r
BF16 = mybir.dt.bfloat16
AF = mybir.ActivationFunctionType
ALU = mybir.AluOpType

# dims
NB = 56
L = NB * NB            # 3136
D = 384                # dim
DC = 3                 # dim chunks of 128
DI = 768               # gru inner
DIC = 6
MLP = 1536
MLPC = 12
B = 2                  # batch per core
NCORES = 8
NT = 392               # time block (= 7 image rows)
NBLK = L // NT         # 8
QT = 784               # scan quarter (= 2 blocks)
NQ = L // QT           # 4
EPS = 1e-5


# ---------------------------------------------------------------- wait fix
def _fix_multiwaits(nc):
    """This walrus accepts at most ONE sync wait per instruction; hoist
    extras into wait-only NoOps on the same engine (streams are in-order)."""
    n = 0
    cnt = [0]
    for f in nc.m.functions:
        for bb in f.blocks:
            out = []
            for inst in bb.instructions:
                si = inst.sync_info
                if si is not None and si.on_wait is not None and len(si.on_wait) > 1:
                    waits = list(si.on_wait)
                    for w in waits[:-1]:
                        cnt[0] += 1
                        nop = mybir.InstNoOp(
                            name=f"I-waitfix-{cnt[0]}",
                            sync_info=mybir.SyncInfo(on_wait=[w], on_update=[]),
                        )
                        nop.engine = inst.engine
                        out.append(nop)
                    inst.sync_info = mybir.SyncInfo(
                        on_wait=[waits[-1]], on_update=list(si.on_update or [])
                    )
                    n += 1
                out.append(inst)
            bb.instructions = out
    return n


# ---------------------------------------------------------------- builder
def _conv_tap_ranges(tap, slab):
    """valid out rows [r0, r1) within image and cols [c0, c1) for tap."""
    dr, dc = tap // 3 - 1, tap % 3 - 1
    rlo, rhi = max(0, -dr), min(NB - 1, NB - 1 - dr)
    r0 = max(7 * slab, rlo)
    r1 = min(7 * slab + 6, rhi)
    c0, c1 = max(0, -dc), min(NB - 1, NB - 1 - dc)
    return dr, dc, r0, r1 + 1, c0, c1 + 1


def build_kernel(reps=1):
    nc = bass.Bass("TRN2", target_bir_lowering=False, debug=False,
                   num_devices=NCORES)

    xT_d = nc.dram_tensor("xT", [B, D, L], BF16, kind="ExternalInput").ap()
    xTf_d = nc.dram_tensor("xTf", [B, D, L], F32, kind="ExternalInput").ap()
    whg1_d = nc.dram_tensor("whg1", [D, 2 * DI], BF16, kind="ExternalInput").ap()
    whg2_d = nc.dram_tensor("whg2", [D, 2 * DI], BF16, kind="ExternalInput").ap()
    wout1_d = nc.dram_tensor("wout1", [DI, D], BF16, kind="ExternalInput").ap()
    wout2_d = nc.dram_tensor("wout2", [DI, D], BF16, kind="ExternalInput").ap()
    p1_d = nc.dram_tensor("p1", [D, MLP], BF16, kind="ExternalInput").ap()
    p2_d = nc.dram_tensor("p2", [MLP, D], BF16, kind="ExternalInput").ap()
    diag_d = nc.dram_tensor("diag", [DC, 128, 9 * 128], BF16,
                            kind="ExternalInput").ap()
    dwb_d = nc.dram_tensor("dwb", [128, DC], F32, kind="ExternalInput").ap()
    p1b_d = nc.dram_tensor("p1b", [128, MLPC], F32, kind="ExternalInput").ap()
    p2b_d = nc.dram_tensor("p2b", [128, DC], F32, kind="ExternalInput").ap()
    out_d = nc.dram_tensor("outT", [B, D, L], F32, kind="ExternalOutput").ap()

    from contextlib import ExitStack
    with tile.TileContext(nc) as tc, ExitStack() as ctx:
        big = ctx.enter_context(tc.tile_pool(name="big", bufs=1))
        wpool = ctx.enter_context(tc.tile_pool(name="wpool", bufs=1))
        work = ctx.enter_context(tc.tile_pool(name="work", bufs=2))
        psum = ctx.enter_context(tc.tile_pool(name="psum", bufs=1, space="PSUM"))
        psumb = ctx.enter_context(tc.tile_pool(name="psumb", bufs=2, space="PSUM"))

        # ---- persistent small constants
        ones_col_f = wpool.tile([128, 1], F32, tag="ones_col_f", name="ones_col_f")
        nc.vector.memset(ones_col_f[:], 1.0)
        ones_col_b = wpool.tile([128, 1], BF16, tag="ones_col_b", name="ones_col_b")
        nc.vector.memset(ones_col_b[:], 1.0)
        ones_col_r = wpool.tile([128, 1], F32R, tag="ones_col_r", name="ones_col_r")
        nc.vector.tensor_copy(ones_col_r[:], ones_col_f[:])
        ones1_b = wpool.tile([1, 128], BF16, tag="ones1_b", name="ones1_b")
        nc.vector.memset(ones1_b[:], 1.0)
        dwb_t = wpool.tile([128, DC], F32, tag="dwb", name="dwb")
        nc.sync.dma_start(dwb_t[:], dwb_d)
        p1b_t = wpool.tile([128, MLPC], F32, tag="p1b", name="p1b")
        nc.sync.dma_start(p1b_t[:], p1b_d)
        p2b_t = wpool.tile([128, DC], F32, tag="p2b", name="p2b")
        nc.sync.dma_start(p2b_t[:], p2b_d)
        eps_t = wpool.tile([1, 1], F32, tag="eps", name="eps")
        nc.vector.memset(eps_t[:], EPS)

        # ---- all weights resident in SBUF (bf16, loaded once)
        def load_w(src_ap, shape, tag):
            t = wpool.tile(shape, BF16, tag=tag, name=tag)
            nc.sync.dma_start(t[:], src_ap)
            return t

        whg_w = [[load_w(d[k * 128:(k + 1) * 128, :], [128, 2 * DI],
                         f"whg{g}_{k}")
                  for k in range(DC)] for g, d in enumerate((whg1_d, whg2_d))]
        wout_w = [[load_w(d[k * 128:(k + 1) * 128, :], [128, D],
                          f"wout{g}_{k}")
                   for k in range(DIC)] for g, d in enumerate((wout1_d, wout2_d))]
        p1w = [load_w(p1_d[k * 128:(k + 1) * 128, :], [128, MLP], f"p1_{k}")
               for k in range(DC)]
        p2w = [load_w(p2_d[k * 128:(k + 1) * 128, :], [128, D], f"p2_{k}")
               for k in range(MLPC)]
        dgw = [load_w(diag_d[c], [128, 9 * 128], f"dg{c}") for c in range(DC)]

        for rep in range(reps):
          for b in range(B):
            # ========================================== phase N1: layernorm1
            x_t = [big.tile([128, L], BF16, tag=f"bufB{c}", name=f"x{c}")
                   for c in range(DC)]
            ob1 = range(NBLK)
            ob2 = range(NBLK - 1, -1, -1)
            for c in range(DC):
                for blk in ob1:
                    bsl = slice(blk * NT, (blk + 1) * NT)
                    nc.sync.dma_start(x_t[c][:, bsl],
                                      xT_d[b, c * 128:(c + 1) * 128, bsl])

            # stat rows: numu/inv as bf16 rows (partition 0) for the
            # broadcast matmuls; m2/ve scratch in strows.
            mmrows = work.tile([1, 2 * L], BF16, tag="mmrows", name="mmrows",
                               bufs=1)
            numu_row = mmrows[0:1, 0:L]
            inv_row = mmrows[0:1, L:2 * L]
            strows = work.tile([97, NT], F32, tag="strows", name="strows",
                               bufs=1)

            def norm_stats(src_mm, sum_ones, order=None):
                """src_mm(c, sl) -> matmul-ready AP (bf16 or f32r)."""
                for blk in (order or range(NBLK)):
                    sl = slice(blk * NT, (blk + 1) * NT)
                    s_ps = psum.tile([1, NT], F32, tag="pA", name="s_ps")
                    q_ps = psum.tile([1, NT], F32, tag="pB", name="q_ps")
                    for c in range(DC):
                        sq = work.tile([128, NT], BF16, tag="sq", name="sq",
                                       bufs=1)
                        src = src_mm(c, sl)
                        nc.scalar.activation(sq[:], src, AF.Square)
                        nc.tensor.matmul(s_ps[:], sum_ones[:], src,
                                         start=(c == 0), stop=(c == DC - 1))
                        nc.tensor.matmul(q_ps[:], ones_col_b[:], sq[:],
                                         start=(c == 0), stop=(c == DC - 1))
                    nc.scalar.activation(numu_row[:, sl], s_ps[:], AF.Copy,
                                         scale=-1.0 / D)
                    m2 = strows[64:65, 0:NT]
                    nc.scalar.activation(m2, s_ps[:], AF.Square, scale=1.0 / D)
                    ve = strows[96:97, 0:NT]
                    nc.vector.scalar_tensor_tensor(ve, q_ps[:], 1.0 / D,
                                                   m2, ALU.mult, ALU.subtract)
                    sd_blk = strows[32:33, 0:NT]
                    nc.scalar.activation(sd_blk, ve, AF.Sqrt, bias=eps_t[:])
                    with nc.allow_low_precision(reason="bf16 norm rows"):
                        nc.vector.reciprocal(inv_row[:, sl], sd_blk)

            def norm_apply(dst_tiles, src_any, order=None):
                """dst = (x + (-mu)) * inv, both rows broadcast via K=1 mms."""
                for blk in (order or range(NBLK)):
                    sl = slice(blk * NT, (blk + 1) * NT)
                    mb_ps = psum.tile([128, NT], F32,
                                      tag=("pC" if blk % 2 == 0 else "pA"),
                                      name="mb_ps")
                    nc.tensor.matmul(mb_ps[:], ones1_b[:], numu_row[:, sl],
                                     start=True, stop=True)
                    ib_ps = psum.tile([128, NT], F32,
                                      tag=("pD" if blk % 2 == 0 else "pB"),
                                      name="ib_ps")
                    nc.tensor.matmul(ib_ps[:], ones1_b[:], inv_row[:, sl],
                                     start=True, stop=True)
                    for c in range(DC):
                        t = work.tile([128, NT], F32, tag="t_ap", name="t_ap",
                                      bufs=2)
                        nc.vector.tensor_tensor(t[:], src_any(c, sl), mb_ps[:],
                                                ALU.add)
                        nc.vector.tensor_tensor(dst_tiles[c][:, sl], t[:],
                                                ib_ps[:], ALU.mult)

            norm_stats(lambda c, sl: x_t[c][:, sl], ones_col_b, order=ob1)
            xh_t = [big.tile([128, L], BF16, tag=f"bufC{c}", name=f"xh{c}")
                    for c in range(DC)]
            norm_apply(xh_t, lambda c, sl: x_t[c][:, sl], order=ob1)

            # ========================================== phase C: dw conv 3x3
            # column shifts via pre-shifted copies so every tap is a
            # row-contiguous 2D slice
            hc_t = [big.tile([128, L], BF16, tag=f"bufB{c}", name=f"hc{c}")
                    for c in range(DC)]
            for c in range(DC):
                dg = dgw[c]
                for slab in ob1:
                    w0 = max(0, 7 * slab - 1)
                    w1 = min(NB, 7 * slab + 8)
                    nw = w1 - w0
                    win = xh_t[c][:, w0 * NB:w1 * NB]
                    xm = work.tile([128, 512], BF16, tag="xsh0", name="xm",
                                   bufs=1)
                    nc.vector.tensor_copy(xm[:, 1:nw * NB], win[:, 0:nw * NB - 1])
                    xm3 = xm[:, 0:nw * NB].rearrange("p (r cc) -> p r cc", cc=NB)
                    nc.vector.tensor_scalar(xm3[:, :, 0:1], xm3[:, :, 0:1],
                                            0.0, None, ALU.mult)
                    xp = work.tile([128, 512], BF16, tag="xsh1", name="xp",
                                   bufs=1)
                    nc.vector.tensor_copy(xp[:, 0:nw * NB - 1], win[:, 1:nw * NB])
                    xp3 = xp[:, 0:nw * NB].rearrange("p (r cc) -> p r cc", cc=NB)
                    nc.vector.tensor_scalar(xp3[:, :, NB - 1:NB],
                                            xp3[:, :, NB - 1:NB],
                                            0.0, None, ALU.mult)
                    cp = psumb.tile([128, NT], F32,
                                    tag=("hp_ps" if slab % 2 == 0 else "gp_ps"),
                                    name="conv_ps")
                    tap_order = [0, 3, 6, 2, 5, 8, 1, 4, 7]
                    for ti, tap in enumerate(tap_order):
                        dr, dcc, r0, r1, c0, c1 = _conv_tap_ranges(tap, slab)
                        osl = slice((r0 - 7 * slab) * NB, (r1 - 7 * slab) * NB)
                        if dcc == 0:
                            rhs = xh_t[c][:, (r0 + dr) * NB:(r1 + dr) * NB]
                        elif dcc == -1:
                            rhs = xm[:, (r0 + dr - w0) * NB:(r1 + dr - w0) * NB]
                        else:
                            rhs = xp[:, (r0 + dr - w0) * NB:(r1 + dr - w0) * NB]
                        nc.tensor.matmul(
                            cp[:, osl], dg[:, tap * 128:(tap + 1) * 128], rhs,
                            start=(ti == 0), stop=(ti == 8))
                    nc.scalar.activation(
                        hc_t[c][:, slab * NT:(slab + 1) * NT], cp[:],
                        AF.Identity, bias=dwb_t[:, c:c + 1])

            # ========================================== phase G: bi-minGRU
            y_t = [big.tile([128, L], F32R, tag=f"bufA{c}", name=f"y{c}")
                   for c in range(DC)]

            for gi, g in enumerate((0, 1)):
                whg = whg_w[g]
                wout = wout_w[g]
                qorder = range(NQ) if g == 0 else range(NQ - 1, -1, -1)
                hs_prev = None
                for qi, q in enumerate(qorder):
                    hs = [work.tile([128, QT], BF16, tag=f"hs{j}",
                                    name=f"hs{j}", bufs=2) for j in range(DIC)]
                    for j in range(DIC):
                        z = work.tile([128, QT], BF16, tag="z", name="z", bufs=2)
                        s = work.tile([128, QT], BF16, tag="s", name="s", bufs=1)
                        for nb2 in range(2):
                            nsl = slice(q * QT + nb2 * NT,
                                        q * QT + (nb2 + 1) * NT)
                            hsl = slice(nb2 * NT, (nb2 + 1) * NT)
                            hp = psumb.tile([128, NT], F32, tag="hp_ps",
                                            name="hp_ps")
                            gp = psumb.tile([128, NT], F32, tag="gp_ps",
                                            name="gp_ps")
                            for k in range(DC):
                                nc.tensor.matmul(
                                    hp[:], whg[k][:, j * 128:(j + 1) * 128],
                                    hc_t[k][:, nsl],
                                    start=(k == 0), stop=(k == DC - 1))
                            for k in range(DC):
                                nc.tensor.matmul(
                                    gp[:],
                                    whg[k][:, DI + j * 128:DI + (j + 1) * 128],
                                    hc_t[k][:, nsl],
                                    start=(k == 0), stop=(k == DC - 1))
                            nc.scalar.activation(z[:, hsl], gp[:], AF.Sigmoid)
                            nc.scalar.activation(s[:, hsl], hp[:], AF.Sigmoid)
                            # g = max(hidden+0.5, sigmoid(hidden)) in place
                            nc.vector.scalar_tensor_tensor(
                                s[:, hsl], hp[:], 0.5, s[:, hsl],
                                ALU.add, ALU.max)
                        bb = work.tile([128, QT], BF16, tag="bb", name="bb",
                                       bufs=2)
                        nc.vector.tensor_tensor(bb[:], z[:], s[:], ALU.mult)
                        # a = 1 - z in place on z (after bb consumed z)
                        nc.gpsimd.tensor_scalar(z[:], z[:], -1.0, 1.0,
                                                ALU.mult, ALU.add)
                        if qi == 0:
                            init = 0.0
                        elif g == 0:
                            init = hs_prev[j][:, QT - 1:QT]
                        else:
                            init = hs_prev[j][:, 0:1]
                        if g == 0:
                            nc.vector.tensor_tensor_scan(
                                hs[j][:], z[:], bb[:], init, ALU.mult, ALU.add)
                        else:
                            rv = slice(None, None, -1)
                            nc.vector.tensor_tensor_scan(
                                hs[j][:, rv], z[:, rv], bb[:, rv], init,
                                ALU.mult, ALU.add)
                    for dc in range(DC):
                        for nb2 in range(2):
                            y_ps = psum.tile(
                                [128, NT], F32,
                                tag=("pC" if (dc * 2 + nb2) % 2 == 0 else "pD"),
                                name="y_ps")
                            for k in range(DIC):
                                nc.tensor.matmul(
                                    y_ps[:], wout[k][:, dc * 128:(dc + 1) * 128],
                                    hs[k][:, nb2 * NT:(nb2 + 1) * NT],
                                    start=(k == 0), stop=(k == DIC - 1))
                            ysl = slice(q * QT + nb2 * NT,
                                        q * QT + (nb2 + 1) * NT)
                            if gi == 0:
                                nc.scalar.activation(y_t[dc][:, ysl], y_ps[:],
                                                     AF.Copy)
                            else:
                                nc.vector.tensor_tensor(
                                    y_t[dc][:, ysl],
                                    y_t[dc][:, ysl].bitcast(F32), y_ps[:],
                                    ALU.add)
                    hs_prev = hs
            # residual: y += x (f32 copy from DRAM)
            for blk in ob2:
                for c in range(DC):
                    sl = slice(blk * NT, (blk + 1) * NT)
                    xr = work.tile([128, NT], F32, tag="xr", name="xr",
                                   bufs=2)
                    nc.sync.dma_start(xr[:], xTf_d[b, c * 128:(c + 1) * 128, sl])
                    nc.vector.tensor_tensor(y_t[c][:, sl],
                                            y_t[c][:, sl].bitcast(F32), xr[:],
                                            ALU.add)

            # ========================================== phase N2: layernorm2
            norm_stats(lambda c, sl: y_t[c][:, sl], ones_col_r, order=ob2)
            yh_t = [big.tile([128, L], BF16, tag=f"bufC{c}", name=f"yh{c}")
                    for c in range(DC)]
            norm_apply(yh_t, lambda c, sl: y_t[c][:, sl].bitcast(F32),
                       order=ob2)

            # ========================================== phase M: MLP (2-pass)
            for blk in ob2:
                sl = slice(blk * NT, (blk + 1) * NT)
                o_ps = []
                for dc in range(DC):
                    if dc < 2:
                        o_ps.append(psum.tile([128, NT], F32,
                                              tag=("pC" if dc == 0 else "pD"),
                                              name=f"o_ps{dc}"))
                    else:
                        o_ps.append(psumb.tile([128, NT], F32, tag="hp_ps",
                                               name="o_ps2"))
                for half in range(2):
                    qs = []
                    for mi in range(6):
                        mc = half * 6 + mi
                        q_ps = psum.tile([128, NT], F32,
                                         tag=("pA" if mi % 2 == 0 else "pB"),
                                         name="q_ps_m")
                        for k in range(DC):
                            nc.tensor.matmul(
                                q_ps[:], p1w[k][:, mc * 128:(mc + 1) * 128],
                                yh_t[k][:, sl],
                                start=(k == 0), stop=(k == DC - 1))
                        qt = work.tile([128, NT], BF16, tag=f"hs{mi}",
                                       name=f"q_sb{mi}", bufs=2)
                        nc.scalar.activation(qt[:], q_ps[:], AF.Gelu,
                                             bias=p1b_t[:, mc:mc + 1])
                        qs.append((mc, qt))
                    for dc in range(DC):
                        for mi, (mc, qt) in enumerate(qs):
                            nc.tensor.matmul(
                                o_ps[dc][:], p2w[mc][:, dc * 128:(dc + 1) * 128],
                                qt[:],
                                start=(half == 0 and mi == 0),
                                stop=(half == 1 and mi == 5))
                for dc in range(DC):
                    oo = work.tile([128, NT], F32, tag="t_ap",
                                   name="oo", bufs=2)
                    nc.vector.scalar_tensor_tensor(
                        oo[:], o_ps[dc][:], p2b_t[:, dc:dc + 1],
                        y_t[dc][:, sl].bitcast(F32), ALU.add, ALU.add)
                    nc.sync.dma_start(
                        out_d[b, dc * 128:(dc + 1) * 128, sl], oo[:])

    return nc


# ---------------------------------------------------------------- host side
_NC_CACHE = {}


def _get_nc():
    key = "bf16"
    if key not in _NC_CACHE:
        nc = build_kernel()
        _fix_multiwaits(nc)
        _NC_CACHE[key] = nc
    return _NC_CACHE[key]


def _prep_weights(inp):
    f = np.float32
    bf = ml_dtypes.bfloat16
    dw_w = np.asarray(inp["dw_w"], f)          # [D,1,3,3]
    norm_w = np.asarray(inp["norm_w"], f)
    norm_b = np.asarray(inp["norm_b"], f)
    dw_wf = dw_w[:, 0] * norm_w[:, None, None]     # [D,3,3]
    dw_bf = np.asarray(inp["dw_b"], f) + norm_b * dw_w[:, 0].sum(axis=(1, 2))
    p1_w = np.asarray(inp["p1_w"], f)
    p1f = p1_w * np.asarray(inp["norm2_w"], f)[:, None]
    p1bf = np.asarray(inp["p1_b"], f) + np.asarray(inp["norm2_b"], f) @ p1_w

    # conv diagonal weight blocks: [DC, 128, 9*128]
    diag = np.zeros((DC, 128, 9 * 128), f)
    ar = np.arange(128)
    for c in range(DC):
        for tap in range(9):
            dr, dcc = tap // 3, tap % 3
            diag[c, ar, tap * 128 + ar] = dw_wf[c * 128:(c + 1) * 128, dr, dcc]

    return dict(
        whg1=np.ascontiguousarray(np.asarray(inp["gru1_whg"], f)).astype(bf),
        whg2=np.ascontiguousarray(np.asarray(inp["gru2_whg"], f)).astype(bf),
        wout1=np.ascontiguousarray(np.asarray(inp["gru1_wout"], f)).astype(bf),
        wout2=np.ascontiguousarray(np.asarray(inp["gru2_wout"], f)).astype(bf),
        p1=np.ascontiguousarray(p1f).astype(bf),
        p2=np.ascontiguousarray(np.asarray(inp["p2_w"], f)).astype(bf),
        diag=diag.astype(bf),
        dwb=np.ascontiguousarray(dw_bf.reshape(DC, 128).T, f),
        p1b=np.ascontiguousarray(p1bf.reshape(MLPC, 128).T, f),
        p2b=np.ascontiguousarray(np.asarray(inp["p2_b"], f).reshape(DC, 128).T, f),
    )


def _build_in_maps(x, w):
    """x: [16, L, D] f32.  Returns per-core input maps."""
    in_maps = []
    for core in range(NCORES):
        xb = x[core * B:(core + 1) * B]                   # [B, L, D]
        xT = np.ascontiguousarray(xb.transpose(0, 2, 1))  # [B, D, L] f32
        m = dict(w)
        m["xTf"] = xT
        m["xT"] = xT.astype(ml_dtypes.bfloat16)
        in_maps.append(m)
    return in_maps


def kernel(**inputs):
    x = np.asarray(inputs["x"], np.float32)    # [16, L, D]
    w = _prep_weights(inputs)
    nc = _get_nc()
    in_maps = _build_in_maps(x, w)
    res = run_bass_kernel_spmd(nc, in_maps, core_ids=list(range(NCORES)))
    outs = []
    for core in range(NCORES):
        oT = res.results[core]["outT"]                    # [B, D, L]
        outs.append(oT.transpose(0, 2, 1))                # [B, L, D]
    return np.ascontiguousarray(np.concatenate(outs, axis=0), np.float32)


# revision 12
# speedup vs baseline: 1.1035x; 1.1035x over previous
"""Trainium2 Bass kernel for nn_Block2DGRU (norm->dwconv3x3->bi-minGRU->norm->MLP).

Self-contained: host-side weight folding + sharding, device kernel via
Bass/Tile, SPMD over 8 NeuronCores (data-parallel over batch: 2 per core).

Device layout: everything [feature_on_partitions, time_on_free].  The minGRU
linear recurrence h_t = a_t*h_{t-1} + b_t runs natively on the DVE via
tensor_tensor_scan (per-partition scan along the free dim); GRU2 is the same
scan with reversed access patterns.

Precision: weights and activations are bf16 (converted on host; 1 cyc/row on
the PE like f32r, but halves SBUF/DMA and doubles 16-bit DVE throughput).
The y accumulator (residual + norm2 input) stays f32r; the x residual is
re-read from an f32 copy in DRAM.  PSUM accumulation is always fp32.
"""
import numpy as np
import ml_dtypes

import concourse.bass as bass
import concourse.tile as tile
import concourse.mybir as mybir
from concourse.bass_utils import run_bass_kernel_spmd

F32 = mybir.dt.float32
F32R = mybir.dt.float32r
BF16 = mybir.dt.bfloat16
AF = mybir.ActivationFunctionType
ALU = mybir.AluOpType

# dims
NB = 56
L = NB * NB            # 3136
D = 384                # dim
DC = 3                 # dim chunks of 128
DI = 768               # gru inner
DIC = 6
MLP = 1536
MLPC = 12
B = 2                  # batch per core
NCORES = 8
NT = 392               # time block (= 7 image rows)
NBLK = L // NT         # 8
QT = 784               # scan quarter (= 2 blocks)
NQ = L // QT           # 4
EPS = 1e-5


# ---------------------------------------------------------------- wait fix
def _fix_multiwaits(nc):
    """This walrus accepts at most ONE sync wait per instruction; hoist
    extras into wait-only NoOps on the same engine (streams are in-order)."""
    n = 0
    cnt = [0]
    for f in nc.m.functions:
        for bb in f.blocks:
            out = []
            for inst in bb.instructions:
                si = inst.sync_info
                if si is not None and si.on_wait is not None and len(si.on_wait) > 1:
                    waits = list(si.on_wait)
                    for w in waits[:-1]:
                        cnt[0] += 1
                        nop = mybir.InstNoOp(
                            name=f"I-waitfix-{cnt[0]}",
                            sync_info=mybir.SyncInfo(on_wait=[w], on_update=[]),
                        )
                        nop.engine = inst.engine
                        out.append(nop)
                    inst.sync_info = mybir.SyncInfo(
                        on_wait=[waits[-1]], on_update=list(si.on_update or [])
                    )
                    n += 1
                out.append(inst)
            bb.instructions = out
    return n


# ---------------------------------------------------------------- builder
def _conv_tap_ranges(tap, slab):
    """valid out rows [r0, r1) within image and cols [c0, c1) for tap."""
    dr, dc = tap // 3 - 1, tap % 3 - 1
    rlo, rhi = max(0, -dr), min(NB - 1, NB - 1 - dr)
    r0 = max(7 * slab, rlo)
    r1 = min(7 * slab + 6, rhi)
    c0, c1 = max(0, -dc), min(NB - 1, NB - 1 - dc)
    return dr, dc, r0, r1 + 1, c0, c1 + 1


def build_kernel(reps=1):
    nc = bass.Bass("TRN2", target_bir_lowering=False, debug=False,
                   num_devices=NCORES)

    xT_d = nc.dram_tensor("xT", [B, D, L], BF16, kind="ExternalInput").ap()
    xTf_d = nc.dram_tensor("xTf", [B, D, L], F32, kind="ExternalInput").ap()
    whg1_d = nc.dram_tensor("whg1", [D, 2 * DI], BF16, kind="ExternalInput").ap()
    whg2_d = nc.dram_tensor("whg2", [D, 2 * DI], BF16, kind="ExternalInput").ap()
    wout1_d = nc.dram_tensor("wout1", [DI, D], BF16, kind="ExternalInput").ap()
    wout2_d = nc.dram_tensor("wout2", [DI, D], BF16, kind="ExternalInput").ap()
    p1_d = nc.dram_tensor("p1", [D, MLP], BF16, kind="ExternalInput").ap()
    p2_d = nc.dram_tensor("p2", [MLP, D], BF16, kind="ExternalInput").ap()
    diag_d = nc.dram_tensor("diag", [DC, 128, 9 * 128], BF16,
                            kind="ExternalInput").ap()
    dwb_d = nc.dram_tensor("dwb", [128, DC], F32, kind="ExternalInput").ap()
    p1b_d = nc.dram_tensor("p1b", [128, MLPC], F32, kind="ExternalInput").ap()
    p2b_d = nc.dram_tensor("p2b", [128, DC], F32, kind="ExternalInput").ap()
    out_d = nc.dram_tensor("outT", [B, D, L], F32, kind="ExternalOutput").ap()

    from contextlib import ExitStack
    with tile.TileContext(nc) as tc, ExitStack() as ctx:
        big = ctx.enter_context(tc.tile_pool(name="big", bufs=1))
        wpool = ctx.enter_context(tc.tile_pool(name="wpool", bufs=1))
        work = ctx.enter_context(tc.tile_pool(name="work", bufs=2))
        psum = ctx.enter_context(tc.tile_pool(name="psum", bufs=1, space="PSUM"))
        psumb = ctx.enter_context(tc.tile_pool(name="psumb", bufs=2, space="PSUM"))

        # ---- persistent small constants
        ones_col_f = wpool.tile([128, 1], F32, tag="ones_col_f", name="ones_col_f")
        nc.vector.memset(ones_col_f[:], 1.0)
        ones_col_b = wpool.tile([128, 1], BF16, tag="ones_col_b", name="ones_col_b")
        nc.vector.memset(ones_col_b[:], 1.0)
        ones_col_r = wpool.tile([128, 1], F32R, tag="ones_col_r", name="ones_col_r")
        nc.vector.tensor_copy(ones_col_r[:], ones_col_f[:])
        ones1_b = wpool.tile([1, 128], BF16, tag="ones1_b", name="ones1_b")
        nc.vector.memset(ones1_b[:], 1.0)
        dwb_t = wpool.tile([128, DC], F32, tag="dwb", name="dwb")
        nc.sync.dma_start(dwb_t[:], dwb_d)
        p1b_t = wpool.tile([128, MLPC], F32, tag="p1b", name="p1b")
        nc.sync.dma_start(p1b_t[:], p1b_d)
        p2b_t = wpool.tile([128, DC], F32, tag="p2b", name="p2b")
        nc.sync.dma_start(p2b_t[:], p2b_d)
        eps_t = wpool.tile([1, 1], F32, tag="eps", name="eps")
        nc.vector.memset(eps_t[:], EPS)

        # ---- all weights resident in SBUF (bf16, loaded once)
        def load_w(src_ap, shape, tag):
            t = wpool.tile(shape, BF16, tag=tag, name=tag)
            nc.sync.dma_start(t[:], src_ap)
            return t

        whg_w = [[load_w(d[k * 128:(k + 1) * 128, :], [128, 2 * DI],
                         f"whg{g}_{k}")
                  for k in range(DC)] for g, d in enumerate((whg1_d, whg2_d))]
        wout_w = [[load_w(d[k * 128:(k + 1) * 128, :], [128, D],
                          f"wout{g}_{k}")
                   for k in range(DIC)] for g, d in enumerate((wout1_d, wout2_d))]
        p1w = [load_w(p1_d[k * 128:(k + 1) * 128, :], [128, MLP], f"p1_{k}")
               for k in range(DC)]
        p2w = [load_w(p2_d[k * 128:(k + 1) * 128, :], [128, D], f"p2_{k}")
               for k in range(MLPC)]
        dgw = [load_w(diag_d[c], [128, 9 * 128], f"dg{c}") for c in range(DC)]

        for rep in range(reps):
          for b in range(B):
            # ========================================== phase N1: layernorm1
            x_t = [big.tile([128, L], BF16, tag=f"bufB{c}", name=f"x{c}")
                   for c in range(DC)]
            ob1 = range(NBLK)
            ob2 = range(NBLK - 1, -1, -1)
            for c in range(DC):
                for blk in ob1:
                    bsl = slice(blk * NT, (blk + 1) * NT)
                    nc.sync.dma_start(x_t[c][:, bsl],
                                      xT_d[b, c * 128:(c + 1) * 128, bsl])

            # stat rows: numu/inv as bf16 rows (partition 0) for the
            # broadcast matmuls; m2/ve scratch in strows.
            mmrows = work.tile([1, 2 * L], BF16, tag="mmrows", name="mmrows",
                               bufs=1)
            numu_row = mmrows[0:1, 0:L]
            inv_row = mmrows[0:1, L:2 * L]
            strows = work.tile([97, NT], F32, tag="strows", name="strows",
                               bufs=1)

            def norm_stats(src_mm, sum_ones, order=None):
                """src_mm(c, sl) -> matmul-ready AP (bf16 or f32r)."""
                for blk in (order or range(NBLK)):
                    sl = slice(blk * NT, (blk + 1) * NT)
                    s_ps = psum.tile([1, NT], F32, tag="pA", name="s_ps")
                    q_ps = psum.tile([1, NT], F32, tag="pB", name="q_ps")
                    for c in range(DC):
                        sq = work.tile([128, NT], F32R, tag="sq", name="sq",
                                       bufs=1)
                        src = src_mm(c, sl)
                        nc.scalar.activation(sq[:], src, AF.Square)
                        nc.tensor.matmul(s_ps[:], sum_ones[:], src,
                                         start=(c == 0), stop=(c == DC - 1))
                        nc.tensor.matmul(q_ps[:], ones_col_r[:], sq[:],
                                         start=(c == 0), stop=(c == DC - 1))
                    nc.scalar.activation(numu_row[:, sl], s_ps[:], AF.Copy,
                                         scale=-1.0 / D)
                    m2 = strows[64:65, 0:NT]
                    nc.scalar.activation(m2, s_ps[:], AF.Square, scale=1.0 / D)
                    ve = strows[96:97, 0:NT]
                    nc.vector.scalar_tensor_tensor(ve, q_ps[:], 1.0 / D,
                                                   m2, ALU.mult, ALU.subtract)
                    sd_blk = strows[32:33, 0:NT]
                    nc.scalar.activation(sd_blk, ve, AF.Sqrt, bias=eps_t[:])
                    pkb = work.tile([7, NB], F32, tag="pk", name="pkb", bufs=2)
                    nc.sync.dma_start(pkb[:], sd_blk)
                    ikb = work.tile([7, NB], BF16, tag="ipk", name="ikb",
                                    bufs=2)
                    with nc.allow_low_precision(reason="bf16 norm rows"):
                        nc.vector.reciprocal(ikb[:], pkb[:])
                    nc.sync.dma_start(inv_row[:, sl], ikb[:])

            def norm_apply(dst_tiles, src_any, order=None):
                """dst = (x + (-mu)) * inv, both rows broadcast via K=1 mms."""
                for blk in (order or range(NBLK)):
                    sl = slice(blk * NT, (blk + 1) * NT)
                    mb_ps = psum.tile([128, NT], F32,
                                      tag=("pC" if blk % 2 == 0 else "pA"),
                                      name="mb_ps")
                    nc.tensor.matmul(mb_ps[:], ones1_b[:], numu_row[:, sl],
                                     start=True, stop=True)
                    ib_ps = psum.tile([128, NT], F32,
                                      tag=("pD" if blk % 2 == 0 else "pB"),
                                      name="ib_ps")
                    nc.tensor.matmul(ib_ps[:], ones1_b[:], inv_row[:, sl],
                                     start=True, stop=True)
                    for c in range(DC):
                        t = work.tile([128, NT], F32, tag="t_ap", name="t_ap",
                                      bufs=2)
                        nc.vector.tensor_tensor(t[:], src_any(c, sl), mb_ps[:],
                                                ALU.add)
                        nc.vector.tensor_tensor(dst_tiles[c][:, sl], t[:],
                                                ib_ps[:], ALU.mult)

            norm_stats(lambda c, sl: x_t[c][:, sl], ones_col_b, order=ob1)
            xh_t = [big.tile([128, L], BF16, tag=f"bufC{c}", name=f"xh{c}")
                    for c in range(DC)]
            norm_apply(xh_t, lambda c, sl: x_t[c][:, sl], order=ob1)

            # ========================================== phase C: dw conv 3x3
            # column shifts via pre-shifted copies so every tap is a
            # row-contiguous 2D slice
            hc_t = [big.tile([128, L], BF16, tag=f"bufB{c}", name=f"hc{c}")
                    for c in range(DC)]
            for c in range(DC):
                dg = dgw[c]
                for slab in ob1:
                    w0 = max(0, 7 * slab - 1)
                    w1 = min(NB, 7 * slab + 8)
                    nw = w1 - w0
                    win = xh_t[c][:, w0 * NB:w1 * NB]
                    xm = work.tile([128, 512], BF16, tag="xsh0", name="xm",
                                   bufs=1)
                    nc.vector.tensor_copy(xm[:, 1:nw * NB], win[:, 0:nw * NB - 1])
                    xm3 = xm[:, 0:nw * NB].rearrange("p (r cc) -> p r cc", cc=NB)
                    nc.vector.tensor_scalar(xm3[:, :, 0:1], xm3[:, :, 0:1],
                                            0.0, None, ALU.mult)
                    xp = work.tile([128, 512], BF16, tag="xsh1", name="xp",
                                   bufs=1)
                    nc.vector.tensor_copy(xp[:, 0:nw * NB - 1], win[:, 1:nw * NB])
                    xp3 = xp[:, 0:nw * NB].rearrange("p (r cc) -> p r cc", cc=NB)
                    nc.vector.tensor_scalar(xp3[:, :, NB - 1:NB],
                                            xp3[:, :, NB - 1:NB],
                                            0.0, None, ALU.mult)
                    cp = psumb.tile([128, NT], F32,
                                    tag=("hp_ps" if slab % 2 == 0 else "gp_ps"),
                                    name="conv_ps")
                    tap_order = [0, 3, 6, 2, 5, 8, 1, 4, 7]
                    for ti, tap in enumerate(tap_order):
                        dr, dcc, r0, r1, c0, c1 = _conv_tap_ranges(tap, slab)
                        osl = slice((r0 - 7 * slab) * NB, (r1 - 7 * slab) * NB)
                        if dcc == 0:
                            rhs = xh_t[c][:, (r0 + dr) * NB:(r1 + dr) * NB]
                        elif dcc == -1:
                            rhs = xm[:, (r0 + dr - w0) * NB:(r1 + dr - w0) * NB]
                        else:
                            rhs = xp[:, (r0 + dr - w0) * NB:(r1 + dr - w0) * NB]
                        nc.tensor.matmul(
                            cp[:, osl], dg[:, tap * 128:(tap + 1) * 128], rhs,
                            start=(ti == 0), stop=(ti == 8))
                    nc.scalar.activation(
                        hc_t[c][:, slab * NT:(slab + 1) * NT], cp[:],
                        AF.Identity, bias=dwb_t[:, c:c + 1])

            # ========================================== phase G: bi-minGRU
            y_t = [big.tile([128, L], F32R, tag=f"bufA{c}", name=f"y{c}")
                   for c in range(DC)]

            for gi, g in enumerate((0, 1)):
                whg = whg_w[g]
                wout = wout_w[g]
                qorder = range(NQ) if g == 0 else range(NQ - 1, -1, -1)
                hs_prev = None
                for qi, q in enumerate(qorder):
                    hs = [work.tile([128, QT], BF16, tag=f"hs{j}",
                                    name=f"hs{j}", bufs=2) for j in range(DIC)]
                    for j in range(DIC):
                        z = work.tile([128, QT], F32, tag="z", name="z", bufs=2)
                        s = work.tile([128, QT], F32, tag="s", name="s", bufs=1)
                        for nb2 in range(2):
                            nsl = slice(q * QT + nb2 * NT,
                                        q * QT + (nb2 + 1) * NT)
                            hsl = slice(nb2 * NT, (nb2 + 1) * NT)
                            hp = psumb.tile([128, NT], F32, tag="hp_ps",
                                            name="hp_ps")
                            gp = psumb.tile([128, NT], F32, tag="gp_ps",
                                            name="gp_ps")
                            for k in range(DC):
                                nc.tensor.matmul(
                                    hp[:], whg[k][:, j * 128:(j + 1) * 128],
                                    hc_t[k][:, nsl],
                                    start=(k == 0), stop=(k == DC - 1))
                            for k in range(DC):
                                nc.tensor.matmul(
                                    gp[:],
                                    whg[k][:, DI + j * 128:DI + (j + 1) * 128],
                                    hc_t[k][:, nsl],
                                    start=(k == 0), stop=(k == DC - 1))
                            nc.scalar.activation(z[:, hsl], gp[:], AF.Sigmoid)
                            nc.scalar.activation(s[:, hsl], hp[:], AF.Sigmoid)
                            # g = max(hidden+0.5, sigmoid(hidden)) in place
                            nc.vector.scalar_tensor_tensor(
                                s[:, hsl], hp[:], 0.5, s[:, hsl],
                                ALU.add, ALU.max)
                        bb = work.tile([128, QT], F32, tag="bb", name="bb",
                                       bufs=2)
                        nc.gpsimd.tensor_tensor(bb[:], z[:], s[:], ALU.mult)
                        # a = 1 - z in place on z (after bb consumed z)
                        nc.gpsimd.tensor_scalar(z[:], z[:], -1.0, 1.0,
                                                ALU.mult, ALU.add)
                        if qi == 0:
                            init = 0.0
                        elif g == 0:
                            init = hs_prev[j][:, QT - 1:QT]
                        else:
                            init = hs_prev[j][:, 0:1]
                        if g == 0:
                            nc.vector.tensor_tensor_scan(
                                hs[j][:], z[:], bb[:], init, ALU.mult, ALU.add)
                        else:
                            rv = slice(None, None, -1)
                            nc.vector.tensor_tensor_scan(
                                hs[j][:, rv], z[:, rv], bb[:, rv], init,
                                ALU.mult, ALU.add)
                    for dc in range(DC):
                        for nb2 in range(2):
                            y_ps = psum.tile(
                                [128, NT], F32,
                                tag=("pC" if (dc * 2 + nb2) % 2 == 0 else "pD"),
                                name="y_ps")
                            for k in range(DIC):
                                nc.tensor.matmul(
                                    y_ps[:], wout[k][:, dc * 128:(dc + 1) * 128],
                                    hs[k][:, nb2 * NT:(nb2 + 1) * NT],
                                    start=(k == 0), stop=(k == DIC - 1))
                            ysl = slice(q * QT + nb2 * NT,
                                        q * QT + (nb2 + 1) * NT)
                            if gi == 0:
                                nc.scalar.activation(y_t[dc][:, ysl], y_ps[:],
                                                     AF.Copy)
                            else:
                                nc.vector.tensor_tensor(
                                    y_t[dc][:, ysl],
                                    y_t[dc][:, ysl].bitcast(F32), y_ps[:],
                                    ALU.add)
                    hs_prev = hs
            # residual: y += x (f32 copy from DRAM)
            for blk in ob2:
                for c in range(DC):
                    sl = slice(blk * NT, (blk + 1) * NT)
                    xr = work.tile([128, NT], F32, tag="xr", name="xr",
                                   bufs=2)
                    nc.sync.dma_start(xr[:], xTf_d[b, c * 128:(c + 1) * 128, sl])
                    nc.vector.tensor_tensor(y_t[c][:, sl],
                                            y_t[c][:, sl].bitcast(F32), xr[:],
                                            ALU.add)

            # ========================================== phase N2: layernorm2
            norm_stats(lambda c, sl: y_t[c][:, sl], ones_col_r, order=ob2)
            yh_t = [big.tile([128, L], BF16, tag=f"bufC{c}", name=f"yh{c}")
                    for c in range(DC)]
            norm_apply(yh_t, lambda c, sl: y_t[c][:, sl].bitcast(F32),
                       order=ob2)

            # ========================================== phase M: MLP (2-pass)
            for blk in ob2:
                sl = slice(blk * NT, (blk + 1) * NT)
                o_ps = []
                for dc in range(DC):
                    if dc < 2:
                        o_ps.append(psum.tile([128, NT], F32,
                                              tag=("pC" if dc == 0 else "pD"),
                                              name=f"o_ps{dc}"))
                    else:
                        o_ps.append(psumb.tile([128, NT], F32, tag="hp_ps",
                                               name="o_ps2"))
                for half in range(2):
                    qs = []
                    for mi in range(6):
                        mc = half * 6 + mi
                        q_ps = psum.tile([128, NT], F32,
                                         tag=("pA" if mi % 2 == 0 else "pB"),
                                         name="q_ps_m")
                        for k in range(DC):
                            nc.tensor.matmul(
                                q_ps[:], p1w[k][:, mc * 128:(mc + 1) * 128],
                                yh_t[k][:, sl],
                                start=(k == 0), stop=(k == DC - 1))
                        qt = work.tile([128, NT], BF16, tag=f"hs{mi}",
                                       name=f"q_sb{mi}", bufs=2)
                        nc.scalar.activation(qt[:], q_ps[:], AF.Gelu,
                                             bias=p1b_t[:, mc:mc + 1])
                        qs.append((mc, qt))
                    for dc in range(DC):
                        for mi, (mc, qt) in enumerate(qs):
                            nc.tensor.matmul(
                                o_ps[dc][:], p2w[mc][:, dc * 128:(dc + 1) * 128],
                                qt[:],
                                start=(half == 0 and mi == 0),
                                stop=(half == 1 and mi == 5))
                for dc in range(DC):
                    oo = work.tile([128, NT], F32, tag="t_ap",
                                   name="oo", bufs=2)
                    nc.vector.scalar_tensor_tensor(
                        oo[:], o_ps[dc][:], p2b_t[:, dc:dc + 1],
                        y_t[dc][:, sl].bitcast(F32), ALU.add, ALU.add)
                    nc.sync.dma_start(
                        out_d[b, dc * 128:(dc + 1) * 128, sl], oo[:])

    return nc


# ---------------------------------------------------------------- host side
_NC_CACHE = {}


def _get_nc():
    key = "bf16"
    if key not in _NC_CACHE:
        nc = build_kernel()
        _fix_multiwaits(nc)
        _NC_CACHE[key] = nc
    return _NC_CACHE[key]


def _prep_weights(inp):
    f = np.float32
    bf = ml_dtypes.bfloat16
    dw_w = np.asarray(inp["dw_w"], f)          # [D,1,3,3]
    norm_w = np.asarray(inp["norm_w"], f)
    norm_b = np.asarray(inp["norm_b"], f)
    dw_wf = dw_w[:, 0] * norm_w[:, None, None]     # [D,3,3]
    dw_bf = np.asarray(inp["dw_b"], f) + norm_b * dw_w[:, 0].sum(axis=(1, 2))
    p1_w = np.asarray(inp["p1_w"], f)
    p1f = p1_w * np.asarray(inp["norm2_w"], f)[:, None]
    p1bf = np.asarray(inp["p1_b"], f) + np.asarray(inp["norm2_b"], f) @ p1_w

    # conv diagonal weight blocks: [DC, 128, 9*128]
    diag = np.zeros((DC, 128, 9 * 128), f)
    ar = np.arange(128)
    for c in range(DC):
        for tap in range(9):
            dr, dcc = tap // 3, tap % 3
            diag[c, ar, tap * 128 + ar] = dw_wf[c * 128:(c + 1) * 128, dr, dcc]

    return dict(
        whg1=np.ascontiguousarray(np.asarray(inp["gru1_whg"], f)).astype(bf),
        whg2=np.ascontiguousarray(np.asarray(inp["gru2_whg"], f)).astype(bf),
        wout1=np.ascontiguousarray(np.asarray(inp["gru1_wout"], f)).astype(bf),
        wout2=np.ascontiguousarray(np.asarray(inp["gru2_wout"], f)).astype(bf),
        p1=np.ascontiguousarray(p1f).astype(bf),
        p2=np.ascontiguousarray(np.asarray(inp["p2_w"], f)).astype(bf),
        diag=diag.astype(bf),
        dwb=np.ascontiguousarray(dw_bf.reshape(DC, 128).T, f),
        p1b=np.ascontiguousarray(p1bf.reshape(MLPC, 128).T, f),
        p2b=np.ascontiguousarray(np.asarray(inp["p2_b"], f).reshape(DC, 128).T, f),
    )


def _build_in_maps(x, w):
    """x: [16, L, D] f32.  Returns per-core input maps."""
    in_maps = []
    for core in range(NCORES):
        xb = x[core * B:(core + 1) * B]                   # [B, L, D]
        xT = np.ascontiguousarray(xb.transpose(0, 2, 1))  # [B, D, L] f32
        m = dict(w)
        m["xTf"] = xT
        m["xT"] = xT.astype(ml_dtypes.bfloat16)
        in_maps.append(m)
    return in_maps


def kernel(**inputs):
    x = np.asarray(inputs["x"], np.float32)    # [16, L, D]
    w = _prep_weights(inputs)
    nc = _get_nc()
    in_maps = _build_in_maps(x, w)
    res = run_bass_kernel_spmd(nc, in_maps, core_ids=list(range(NCORES)))
    outs = []
    for core in range(NCORES):
        oT = res.results[core]["outT"]                    # [B, D, L]
        outs.append(oT.transpose(0, 2, 1))                # [B, L, D]
    return np.ascontiguousarray(np.concatenate(outs, axis=0), np.float32)


# revision 16
# speedup vs baseline: 1.1239x; 1.0185x over previous
"""Trainium2 Bass kernel for nn_Block2DGRU (norm->dwconv3x3->bi-minGRU->norm->MLP).

Self-contained: host-side weight folding + sharding, device kernel via
Bass/Tile, SPMD over 8 NeuronCores (data-parallel over batch: 2 per core).

Device layout: everything [feature_on_partitions, time_on_free].  The minGRU
linear recurrence h_t = a_t*h_{t-1} + b_t runs natively on the DVE via
tensor_tensor_scan (per-partition scan along the free dim); GRU2 is the same
scan with reversed access patterns.

Precision: weights and activations are bf16 (converted on host; 1 cyc/row on
the PE like f32r, but halves SBUF/DMA and doubles 16-bit DVE throughput).
The y accumulator (residual + norm2 input) stays f32r; the x residual is
re-read from an f32 copy in DRAM.  PSUM accumulation is always fp32.
"""
import numpy as np
import ml_dtypes

import concourse.bass as bass
import concourse.tile as tile
import concourse.mybir as mybir
from concourse.bass_utils import run_bass_kernel_spmd

F32 = mybir.dt.float32
F32R = mybir.dt.float32r
BF16 = mybir.dt.bfloat16
AF = mybir.ActivationFunctionType
ALU = mybir.AluOpType

# dims
NB = 56
L = NB * NB            # 3136
D = 384                # dim
DC = 3                 # dim chunks of 128
DI = 768               # gru inner
DIC = 6
MLP = 1536
MLPC = 12
B = 2                  # batch per core
NCORES = 8
NT = 392               # time block (= 7 image rows)
NBLK = L // NT         # 8
QT = 784               # scan quarter (= 2 blocks)
NQ = L // QT           # 4
EPS = 1e-5


# ---------------------------------------------------------------- wait fix
def _fix_multiwaits(nc):
    """This walrus accepts at most ONE sync wait per instruction; hoist
    extras into wait-only NoOps on the same engine (streams are in-order)."""
    n = 0
    cnt = [0]
    for f in nc.m.functions:
        for bb in f.blocks:
            out = []
            for inst in bb.instructions:
                si = inst.sync_info
                if si is not None and si.on_wait is not None and len(si.on_wait) > 1:
                    waits = list(si.on_wait)
                    for w in waits[:-1]:
                        cnt[0] += 1
                        nop = mybir.InstNoOp(
                            name=f"I-waitfix-{cnt[0]}",
                            sync_info=mybir.SyncInfo(on_wait=[w], on_update=[]),
                        )
                        nop.engine = inst.engine
                        out.append(nop)
                    inst.sync_info = mybir.SyncInfo(
                        on_wait=[waits[-1]], on_update=list(si.on_update or [])
                    )
                    n += 1
                out.append(inst)
            bb.instructions = out
    return n


# ---------------------------------------------------------------- builder
def _conv_tap_ranges(tap, slab):
    """valid out rows [r0, r1) within image and cols [c0, c1) for tap."""
    dr, dc = tap // 3 - 1, tap % 3 - 1
    rlo, rhi = max(0, -dr), min(NB - 1, NB - 1 - dr)
    r0 = max(7 * slab, rlo)
    r1 = min(7 * slab + 6, rhi)
    c0, c1 = max(0, -dc), min(NB - 1, NB - 1 - dc)
    return dr, dc, r0, r1 + 1, c0, c1 + 1


def build_kernel(reps=1):
    nc = bass.Bass("TRN2", target_bir_lowering=False, debug=False,
                   num_devices=NCORES)

    xT_d = nc.dram_tensor("xT", [B, D, L], BF16, kind="ExternalInput").ap()
    xTf_d = nc.dram_tensor("xTf", [B, D, L], F32, kind="ExternalInput").ap()
    whg1_d = nc.dram_tensor("whg1", [D, 2 * DI], BF16, kind="ExternalInput").ap()
    whg2_d = nc.dram_tensor("whg2", [D, 2 * DI], BF16, kind="ExternalInput").ap()
    wout1_d = nc.dram_tensor("wout1", [DI, D], BF16, kind="ExternalInput").ap()
    wout2_d = nc.dram_tensor("wout2", [DI, D], BF16, kind="ExternalInput").ap()
    p1_d = nc.dram_tensor("p1", [D, MLP], BF16, kind="ExternalInput").ap()
    p2_d = nc.dram_tensor("p2", [MLP, D], BF16, kind="ExternalInput").ap()
    diag_d = nc.dram_tensor("diag", [DC, 128, 9 * 128], BF16,
                            kind="ExternalInput").ap()
    dwb_d = nc.dram_tensor("dwb", [128, DC], F32, kind="ExternalInput").ap()
    p1b_d = nc.dram_tensor("p1b", [128, MLPC], F32, kind="ExternalInput").ap()
    p2b_d = nc.dram_tensor("p2b", [128, DC], F32, kind="ExternalInput").ap()
    out_d = nc.dram_tensor("outT", [B, D, L], F32, kind="ExternalOutput").ap()

    from contextlib import ExitStack
    with tile.TileContext(nc) as tc, ExitStack() as ctx:
        big = ctx.enter_context(tc.tile_pool(name="big", bufs=1))
        wpool = ctx.enter_context(tc.tile_pool(name="wpool", bufs=1))
        work = ctx.enter_context(tc.tile_pool(name="work", bufs=2))
        psum = ctx.enter_context(tc.tile_pool(name="psum", bufs=1, space="PSUM"))
        psumb = ctx.enter_context(tc.tile_pool(name="psumb", bufs=2, space="PSUM"))

        # ---- persistent small constants
        ones_col_f = wpool.tile([128, 1], F32, tag="ones_col_f", name="ones_col_f")
        nc.vector.memset(ones_col_f[:], 1.0)
        ones_col_b = wpool.tile([128, 1], BF16, tag="ones_col_b", name="ones_col_b")
        nc.vector.memset(ones_col_b[:], 1.0)
        ones_col_r = wpool.tile([128, 1], F32R, tag="ones_col_r", name="ones_col_r")
        nc.vector.tensor_copy(ones_col_r[:], ones_col_f[:])
        ones1_b = wpool.tile([1, 128], BF16, tag="ones1_b", name="ones1_b")
        nc.vector.memset(ones1_b[:], 1.0)
        dwb_t = wpool.tile([128, DC], F32, tag="dwb", name="dwb")
        nc.sync.dma_start(dwb_t[:], dwb_d)
        p1b_t = wpool.tile([128, MLPC], F32, tag="p1b", name="p1b")
        nc.sync.dma_start(p1b_t[:], p1b_d)
        p2b_t = wpool.tile([128, DC], F32, tag="p2b", name="p2b")
        nc.sync.dma_start(p2b_t[:], p2b_d)
        eps_t = wpool.tile([1, 1], F32, tag="eps", name="eps")
        nc.vector.memset(eps_t[:], EPS)

        # ---- all weights resident in SBUF (bf16, loaded once)
        def load_w(src_ap, shape, tag):
            t = wpool.tile(shape, BF16, tag=tag, name=tag)
            nc.sync.dma_start(t[:], src_ap)
            return t

        whg_w = [[load_w(d[k * 128:(k + 1) * 128, :], [128, 2 * DI],
                         f"whg{g}_{k}")
                  for k in range(DC)] for g, d in enumerate((whg1_d, whg2_d))]
        wout_w = [[load_w(d[k * 128:(k + 1) * 128, :], [128, D],
                          f"wout{g}_{k}")
                   for k in range(DIC)] for g, d in enumerate((wout1_d, wout2_d))]
        p1w = [load_w(p1_d[k * 128:(k + 1) * 128, :], [128, MLP], f"p1_{k}")
               for k in range(DC)]
        p2w = [load_w(p2_d[k * 128:(k + 1) * 128, :], [128, D], f"p2_{k}")
               for k in range(MLPC)]
        dgw = [load_w(diag_d[c], [128, 9 * 128], f"dg{c}") for c in range(DC)]

        for rep in range(reps):
          for b in range(B):
            # ========================================== phase N1: layernorm1
            x_t = [big.tile([128, L], BF16, tag=f"bufB{c}", name=f"x{c}")
                   for c in range(DC)]
            ob1 = range(NBLK)
            ob2 = range(NBLK - 1, -1, -1)
            for c in range(DC):
                for blk in ob1:
                    bsl = slice(blk * NT, (blk + 1) * NT)
                    nc.sync.dma_start(x_t[c][:, bsl],
                                      xT_d[b, c * 128:(c + 1) * 128, bsl])

            # stat rows: numu/inv as bf16 rows (partition 0) for the
            # broadcast matmuls; m2/ve scratch in strows.
            mmrows = work.tile([1, 2 * L], BF16, tag="mmrows", name="mmrows",
                               bufs=1)
            numu_row = mmrows[0:1, 0:L]
            inv_row = mmrows[0:1, L:2 * L]
            strows = work.tile([97, NT], F32, tag="strows", name="strows",
                               bufs=1)

            def norm_stats(src_mm, sum_ones, order=None):
                """src_mm(c, sl) -> matmul-ready AP (bf16 or f32r)."""
                for blk in (order or range(NBLK)):
                    sl = slice(blk * NT, (blk + 1) * NT)
                    s_ps = psum.tile([1, NT], F32, tag="pA", name="s_ps")
                    q_ps = psum.tile([1, NT], F32, tag="pB", name="q_ps")
                    for c in range(DC):
                        sq = work.tile([128, NT], F32R, tag="sq", name="sq",
                                       bufs=1)
                        src = src_mm(c, sl)
                        nc.scalar.activation(sq[:], src, AF.Square)
                        nc.tensor.matmul(s_ps[:], sum_ones[:], src,
                                         start=(c == 0), stop=(c == DC - 1))
                        nc.tensor.matmul(q_ps[:], ones_col_r[:], sq[:],
                                         start=(c == 0), stop=(c == DC - 1))
                    nc.scalar.activation(numu_row[:, sl], s_ps[:], AF.Copy,
                                         scale=-1.0 / D)
                    m2 = strows[64:65, 0:NT]
                    nc.scalar.activation(m2, s_ps[:], AF.Square, scale=1.0 / D)
                    ve = strows[96:97, 0:NT]
                    nc.vector.scalar_tensor_tensor(ve, q_ps[:], 1.0 / D,
                                                   m2, ALU.mult, ALU.subtract)
                    sd_blk = strows[32:33, 0:NT]
                    nc.scalar.activation(sd_blk, ve, AF.Sqrt, bias=eps_t[:])
                    pkb = work.tile([7, NB], F32, tag="pk", name="pkb", bufs=2)
                    nc.sync.dma_start(pkb[:], sd_blk)
                    ikb = work.tile([7, NB], BF16, tag="ipk", name="ikb",
                                    bufs=2)
                    with nc.allow_low_precision(reason="bf16 norm rows"):
                        nc.vector.reciprocal(ikb[:], pkb[:])
                    nc.sync.dma_start(inv_row[:, sl], ikb[:])

            def norm_apply(dst_tiles, src_any, order=None):
                """dst = (x + (-mu)) * inv, both rows broadcast via K=1 mms."""
                for blk in (order or range(NBLK)):
                    sl = slice(blk * NT, (blk + 1) * NT)
                    mb_ps = psum.tile([128, NT], F32,
                                      tag=("pC" if blk % 2 == 0 else "pA"),
                                      name="mb_ps")
                    nc.tensor.matmul(mb_ps[:], ones1_b[:], numu_row[:, sl],
                                     start=True, stop=True)
                    ib_ps = psum.tile([128, NT], F32,
                                      tag=("pD" if blk % 2 == 0 else "pB"),
                                      name="ib_ps")
                    nc.tensor.matmul(ib_ps[:], ones1_b[:], inv_row[:, sl],
                                     start=True, stop=True)
                    for c in range(DC):
                        t = work.tile([128, NT], F32, tag="t_ap", name="t_ap",
                                      bufs=2)
                        nc.vector.tensor_tensor(t[:], src_any(c, sl), mb_ps[:],
                                                ALU.add)
                        nc.vector.tensor_tensor(dst_tiles[c][:, sl], t[:],
                                                ib_ps[:], ALU.mult)

            norm_stats(lambda c, sl: x_t[c][:, sl], ones_col_b, order=ob1)
            xh_t = [big.tile([128, L], BF16, tag=f"bufC{c}", name=f"xh{c}")
                    for c in range(DC)]
            norm_apply(xh_t, lambda c, sl: x_t[c][:, sl], order=ob1)

            # ========================================== phase C: dw conv 3x3
            # column shifts via pre-shifted copies so every tap is a
            # row-contiguous 2D slice
            hc_t = [big.tile([128, L], BF16, tag=f"bufB{c}", name=f"hc{c}")
                    for c in range(DC)]
            for c in range(DC):
                dg = dgw[c]
                for slab in ob1:
                    w0 = max(0, 7 * slab - 1)
                    w1 = min(NB, 7 * slab + 8)
                    nw = w1 - w0
                    win = xh_t[c][:, w0 * NB:w1 * NB]
                    xm = work.tile([128, 512], BF16, tag="xsh0", name="xm",
                                   bufs=2)
                    nc.vector.tensor_copy(xm[:, 1:nw * NB], win[:, 0:nw * NB - 1])
                    xm3 = xm[:, 0:nw * NB].rearrange("p (r cc) -> p r cc", cc=NB)
                    nc.vector.tensor_scalar(xm3[:, :, 0:1], xm3[:, :, 0:1],
                                            0.0, None, ALU.mult)
                    xp = work.tile([128, 512], BF16, tag="xsh1", name="xp",
                                   bufs=2)
                    nc.vector.tensor_copy(xp[:, 0:nw * NB - 1], win[:, 1:nw * NB])
                    xp3 = xp[:, 0:nw * NB].rearrange("p (r cc) -> p r cc", cc=NB)
                    nc.vector.tensor_scalar(xp3[:, :, NB - 1:NB],
                                            xp3[:, :, NB - 1:NB],
                                            0.0, None, ALU.mult)
                    cp = psumb.tile([128, NT], F32,
                                    tag=("hp_ps" if slab % 2 == 0 else "gp_ps"),
                                    name="conv_ps")
                    tap_order = [0, 3, 6, 2, 5, 8, 1, 4, 7]
                    for ti, tap in enumerate(tap_order):
                        dr, dcc, r0, r1, c0, c1 = _conv_tap_ranges(tap, slab)
                        osl = slice((r0 - 7 * slab) * NB, (r1 - 7 * slab) * NB)
                        if dcc == 0:
                            rhs = xh_t[c][:, (r0 + dr) * NB:(r1 + dr) * NB]
                        elif dcc == -1:
                            rhs = xm[:, (r0 + dr - w0) * NB:(r1 + dr - w0) * NB]
                        else:
                            rhs = xp[:, (r0 + dr - w0) * NB:(r1 + dr - w0) * NB]
                        nc.tensor.matmul(
                            cp[:, osl], dg[:, tap * 128:(tap + 1) * 128], rhs,
                            start=(ti == 0), stop=(ti == 8))
                    nc.scalar.activation(
                        hc_t[c][:, slab * NT:(slab + 1) * NT], cp[:],
                        AF.Identity, bias=dwb_t[:, c:c + 1])

            # ========================================== phase G: bi-minGRU
            y_t = [big.tile([128, L], F32R, tag=f"bufA{c}", name=f"y{c}")
                   for c in range(DC)]

            # flat (direction, quarter) pipeline; out-projections are deferred
            # by one unit so the PE's in-order stream never fences on a scan.
            units = [(0, qi, q) for qi, q in enumerate(range(NQ))] + \
                    [(1, qi, q) for qi, q in enumerate(range(NQ - 1, -1, -1))]

            def unit_out(g, q, hs):
                wout = wout_w[g]
                for dc in range(DC):
                    for nb2 in range(2):
                        y_ps = psum.tile(
                            [128, NT], F32,
                            tag=("pC" if (dc * 2 + nb2) % 2 == 0 else "pD"),
                            name="y_ps")
                        for k in range(DIC):
                            nc.tensor.matmul(
                                y_ps[:], wout[k][:, dc * 128:(dc + 1) * 128],
                                hs[k][:, nb2 * NT:(nb2 + 1) * NT],
                                start=(k == 0), stop=(k == DIC - 1))
                        ysl = slice(q * QT + nb2 * NT,
                                    q * QT + (nb2 + 1) * NT)
                        if g == 0:
                            nc.scalar.activation(y_t[dc][:, ysl], y_ps[:],
                                                 AF.Copy)
                        else:
                            nc.vector.tensor_tensor(
                                y_t[dc][:, ysl],
                                y_t[dc][:, ysl].bitcast(F32), y_ps[:],
                                ALU.add)

            hs_prev = None
            pend = None          # (g, q, hs) awaiting out-projection
            for g, qi, q in units:
                whg = whg_w[g]
                hs = [work.tile([128, QT], BF16, tag=f"hs{j}",
                                name=f"hs{j}", bufs=2) for j in range(DIC)]
                for j in range(DIC):
                    z = work.tile([128, QT], F32, tag="z", name="z", bufs=2)
                    s = work.tile([128, QT], F32, tag="s", name="s", bufs=1)
                    for nb2 in range(2):
                        nsl = slice(q * QT + nb2 * NT,
                                    q * QT + (nb2 + 1) * NT)
                        hsl = slice(nb2 * NT, (nb2 + 1) * NT)
                        hp = psumb.tile([128, NT], F32, tag="hp_ps",
                                        name="hp_ps")
                        gp = psumb.tile([128, NT], F32, tag="gp_ps",
                                        name="gp_ps")
                        for k in range(DC):
                            nc.tensor.matmul(
                                hp[:], whg[k][:, j * 128:(j + 1) * 128],
                                hc_t[k][:, nsl],
                                start=(k == 0), stop=(k == DC - 1))
                        for k in range(DC):
                            nc.tensor.matmul(
                                gp[:],
                                whg[k][:, DI + j * 128:DI + (j + 1) * 128],
                                hc_t[k][:, nsl],
                                start=(k == 0), stop=(k == DC - 1))
                        nc.scalar.activation(z[:, hsl], gp[:], AF.Sigmoid)
                        nc.scalar.activation(s[:, hsl], hp[:], AF.Sigmoid)
                        # g = max(hidden+0.5, sigmoid(hidden)) in place
                        nc.vector.scalar_tensor_tensor(
                            s[:, hsl], hp[:], 0.5, s[:, hsl],
                            ALU.add, ALU.max)
                    bb = work.tile([128, QT], F32, tag="bb", name="bb",
                                   bufs=2)
                    nc.gpsimd.tensor_tensor(bb[:], z[:], s[:], ALU.mult)
                    # a = 1 - z in place on z (after bb consumed z)
                    nc.gpsimd.tensor_scalar(z[:], z[:], -1.0, 1.0,
                                            ALU.mult, ALU.add)
                    if qi == 0:
                        init = 0.0
                    elif g == 0:
                        init = hs_prev[j][:, QT - 1:QT]
                    else:
                        init = hs_prev[j][:, 0:1]
                    if g == 0:
                        nc.vector.tensor_tensor_scan(
                            hs[j][:], z[:], bb[:], init,
                            ALU.mult, ALU.add)
                    else:
                        rv = slice(None, None, -1)
                        nc.vector.tensor_tensor_scan(
                            hs[j][:, rv], z[:, rv], bb[:, rv], init,
                            ALU.mult, ALU.add)
                if pend is not None:
                    unit_out(*pend)
                hs_prev = hs if qi < NQ - 1 else None
                pend = (g, q, hs)
            unit_out(*pend)
            # residual: y += x (f32 copy from DRAM)
            for blk in ob2:
                for c in range(DC):
                    sl = slice(blk * NT, (blk + 1) * NT)
                    xr = work.tile([128, NT], F32, tag="xr", name="xr",
                                   bufs=2)
                    nc.sync.dma_start(xr[:], xTf_d[b, c * 128:(c + 1) * 128, sl])
                    nc.vector.tensor_tensor(y_t[c][:, sl],
                                            y_t[c][:, sl].bitcast(F32), xr[:],
                                            ALU.add)

            # ========================================== phase N2: layernorm2
            norm_stats(lambda c, sl: y_t[c][:, sl], ones_col_r, order=ob2)
            yh_t = [big.tile([128, L], BF16, tag=f"bufC{c}", name=f"yh{c}")
                    for c in range(DC)]
            norm_apply(yh_t, lambda c, sl: y_t[c][:, sl].bitcast(F32),
                       order=ob2)

            # ========================================== phase M: MLP (2-pass)
            for blk in ob2:
                sl = slice(blk * NT, (blk + 1) * NT)
                o_ps = []
                for dc in range(DC):
                    if dc < 2:
                        o_ps.append(psum.tile([128, NT], F32,
                                              tag=("pC" if dc == 0 else "pD"),
                                              name=f"o_ps{dc}"))
                    else:
                        o_ps.append(psumb.tile([128, NT], F32, tag="hp_ps",
                                               name="o_ps2"))
                qs = []
                for mc in range(MLPC):
                    q_ps = psum.tile([128, NT], F32,
                                     tag=("pA" if mc % 2 == 0 else "pB"),
                                     name="q_ps_m")
                    for k in range(DC):
                        nc.tensor.matmul(
                            q_ps[:], p1w[k][:, mc * 128:(mc + 1) * 128],
                            yh_t[k][:, sl],
                            start=(k == 0), stop=(k == DC - 1))
                    qt = work.tile([128, NT], BF16, tag=f"hs{mc % 6}",
                                   name=f"q_sb{mc}", bufs=2)
                    nc.scalar.activation(qt[:], q_ps[:], AF.Gelu,
                                         bias=p1b_t[:, mc:mc + 1])
                    qs.append(qt)
                for dc in range(DC):
                    for mc in range(MLPC):
                        nc.tensor.matmul(
                            o_ps[dc][:], p2w[mc][:, dc * 128:(dc + 1) * 128],
                            qs[mc][:],
                            start=(mc == 0), stop=(mc == MLPC - 1))
                for dc in range(DC):
                    oo = work.tile([128, NT], F32, tag="t_ap",
                                   name="oo", bufs=2)
                    nc.vector.scalar_tensor_tensor(
                        oo[:], o_ps[dc][:], p2b_t[:, dc:dc + 1],
                        y_t[dc][:, sl].bitcast(F32), ALU.add, ALU.add)
                    nc.sync.dma_start(
                        out_d[b, dc * 128:(dc + 1) * 128, sl], oo[:])

    return nc


# ---------------------------------------------------------------- host side
_NC_CACHE = {}


def _get_nc():
    key = "bf16"
    if key not in _NC_CACHE:
        nc = build_kernel()
        _fix_multiwaits(nc)
        _NC_CACHE[key] = nc
    return _NC_CACHE[key]


def _prep_weights(inp):
    f = np.float32
    bf = ml_dtypes.bfloat16
    dw_w = np.asarray(inp["dw_w"], f)          # [D,1,3,3]
    norm_w = np.asarray(inp["norm_w"], f)
    norm_b = np.asarray(inp["norm_b"], f)
    dw_wf = dw_w[:, 0] * norm_w[:, None, None]     # [D,3,3]
    dw_bf = np.asarray(inp["dw_b"], f) + norm_b * dw_w[:, 0].sum(axis=(1, 2))
    p1_w = np.asarray(inp["p1_w"], f)
    p1f = p1_w * np.asarray(inp["norm2_w"], f)[:, None]
    p1bf = np.asarray(inp["p1_b"], f) + np.asarray(inp["norm2_b"], f) @ p1_w

    # conv diagonal weight blocks: [DC, 128, 9*128]
    diag = np.zeros((DC, 128, 9 * 128), f)
    ar = np.arange(128)
    for c in range(DC):
        for tap in range(9):
            dr, dcc = tap // 3, tap % 3
            diag[c, ar, tap * 128 + ar] = dw_wf[c * 128:(c + 1) * 128, dr, dcc]

    return dict(
        whg1=np.ascontiguousarray(np.asarray(inp["gru1_whg"], f)).astype(bf),
        whg2=np.ascontiguousarray(np.asarray(inp["gru2_whg"], f)).astype(bf),
        wout1=np.ascontiguousarray(np.asarray(inp["gru1_wout"], f)).astype(bf),
        wout2=np.ascontiguousarray(np.asarray(inp["gru2_wout"], f)).astype(bf),
        p1=np.ascontiguousarray(p1f).astype(bf),
        p2=np.ascontiguousarray(np.asarray(inp["p2_w"], f)).astype(bf),
        diag=diag.astype(bf),
        dwb=np.ascontiguousarray(dw_bf.reshape(DC, 128).T, f),
        p1b=np.ascontiguousarray(p1bf.reshape(MLPC, 128).T, f),
        p2b=np.ascontiguousarray(np.asarray(inp["p2_b"], f).reshape(DC, 128).T, f),
    )


def _build_in_maps(x, w):
    """x: [16, L, D] f32.  Returns per-core input maps."""
    in_maps = []
    for core in range(NCORES):
        xb = x[core * B:(core + 1) * B]                   # [B, L, D]
        xT = np.ascontiguousarray(xb.transpose(0, 2, 1))  # [B, D, L] f32
        m = dict(w)
        m["xTf"] = xT
        m["xT"] = xT.astype(ml_dtypes.bfloat16)
        in_maps.append(m)
    return in_maps


def kernel(**inputs):
    x = np.asarray(inputs["x"], np.float32)    # [16, L, D]
    w = _prep_weights(inputs)
    nc = _get_nc()
    in_maps = _build_in_maps(x, w)
    res = run_bass_kernel_spmd(nc, in_maps, core_ids=list(range(NCORES)))
    outs = []
    for core in range(NCORES):
        oT = res.results[core]["outT"]                    # [B, D, L]
        outs.append(oT.transpose(0, 2, 1))                # [B, L, D]
    return np.ascontiguousarray(np.concatenate(outs, axis=0), np.float32)


# revision 19
# speedup vs baseline: 1.1637x; 1.0354x over previous
"""Trainium2 Bass kernel for nn_Block2DGRU (norm->dwconv3x3->bi-minGRU->norm->MLP).

Self-contained: host-side weight folding + sharding, device kernel via
Bass/Tile, SPMD over 8 NeuronCores (data-parallel over batch: 2 per core).

Device layout: everything [feature_on_partitions, time_on_free].  The minGRU
linear recurrence h_t = a_t*h_{t-1} + b_t runs natively on the DVE via
tensor_tensor_scan (per-partition scan along the free dim); GRU2 is the same
scan with reversed access patterns.

Precision: weights and activations are bf16 (converted on host; 1 cyc/row on
the PE like f32r, but halves SBUF/DMA and doubles 16-bit DVE throughput).
The y accumulator (residual + norm2 input) stays f32r; the x residual is
re-read from an f32 copy in DRAM.  PSUM accumulation is always fp32.
"""
import numpy as np
import ml_dtypes

import concourse.bass as bass
import concourse.tile as tile
import concourse.mybir as mybir
from concourse.bass_utils import run_bass_kernel_spmd

F32 = mybir.dt.float32
F32R = mybir.dt.float32r
BF16 = mybir.dt.bfloat16
AF = mybir.ActivationFunctionType
ALU = mybir.AluOpType

# dims
NB = 56
L = NB * NB            # 3136
D = 384                # dim
DC = 3                 # dim chunks of 128
DI = 768               # gru inner
DIC = 6
MLP = 1536
MLPC = 12
B = 2                  # batch per core
NCORES = 8
NT = 392               # time block (= 7 image rows)
NBLK = L // NT         # 8
QT = 784               # scan quarter (= 2 blocks)
NQ = L // QT           # 4
EPS = 1e-5


# ---------------------------------------------------------------- wait fix
def _fix_multiwaits(nc):
    """This walrus accepts at most ONE sync wait per instruction; hoist
    extras into wait-only NoOps on the same engine (streams are in-order)."""
    n = 0
    cnt = [0]
    for f in nc.m.functions:
        for bb in f.blocks:
            out = []
            for inst in bb.instructions:
                si = inst.sync_info
                if si is not None and si.on_wait is not None and len(si.on_wait) > 1:
                    waits = list(si.on_wait)
                    for w in waits[:-1]:
                        cnt[0] += 1
                        nop = mybir.InstNoOp(
                            name=f"I-waitfix-{cnt[0]}",
                            sync_info=mybir.SyncInfo(on_wait=[w], on_update=[]),
                        )
                        nop.engine = inst.engine
                        out.append(nop)
                    inst.sync_info = mybir.SyncInfo(
                        on_wait=[waits[-1]], on_update=list(si.on_update or [])
                    )
                    n += 1
                out.append(inst)
            bb.instructions = out
    return n


# ---------------------------------------------------------------- builder
def _conv_tap_ranges(tap, slab):
    """valid out rows [r0, r1) within image and cols [c0, c1) for tap."""
    dr, dc = tap // 3 - 1, tap % 3 - 1
    rlo, rhi = max(0, -dr), min(NB - 1, NB - 1 - dr)
    r0 = max(7 * slab, rlo)
    r1 = min(7 * slab + 6, rhi)
    c0, c1 = max(0, -dc), min(NB - 1, NB - 1 - dc)
    return dr, dc, r0, r1 + 1, c0, c1 + 1


def build_kernel(reps=1):
    nc = bass.Bass("TRN2", target_bir_lowering=False, debug=False,
                   num_devices=NCORES)

    xT_d = nc.dram_tensor("xT", [B, D, L], BF16, kind="ExternalInput").ap()
    xTf_d = nc.dram_tensor("xTf", [B, D, L], F32, kind="ExternalInput").ap()
    whg1_d = nc.dram_tensor("whg1", [D, 2 * DI], BF16, kind="ExternalInput").ap()
    whg2_d = nc.dram_tensor("whg2", [D, 2 * DI], BF16, kind="ExternalInput").ap()
    wout1_d = nc.dram_tensor("wout1", [DI, D], BF16, kind="ExternalInput").ap()
    wout2_d = nc.dram_tensor("wout2", [DI, D], BF16, kind="ExternalInput").ap()
    p1_d = nc.dram_tensor("p1", [D, MLP], BF16, kind="ExternalInput").ap()
    p2_d = nc.dram_tensor("p2", [MLP, D], BF16, kind="ExternalInput").ap()
    diag_d = nc.dram_tensor("diag", [DC, 128, 9 * 128], BF16,
                            kind="ExternalInput").ap()
    dwb_d = nc.dram_tensor("dwb", [128, DC], F32, kind="ExternalInput").ap()
    p1b_d = nc.dram_tensor("p1b", [128, MLPC], F32, kind="ExternalInput").ap()
    p2b_d = nc.dram_tensor("p2b", [128, DC], F32, kind="ExternalInput").ap()
    out_d = nc.dram_tensor("outT", [B, D, L], F32, kind="ExternalOutput").ap()

    from contextlib import ExitStack
    with tile.TileContext(nc) as tc, ExitStack() as ctx:
        big = ctx.enter_context(tc.tile_pool(name="big", bufs=1))
        wpool = ctx.enter_context(tc.tile_pool(name="wpool", bufs=1))
        work = ctx.enter_context(tc.tile_pool(name="work", bufs=2))
        psum = ctx.enter_context(tc.tile_pool(name="psum", bufs=1, space="PSUM"))
        psumb = ctx.enter_context(tc.tile_pool(name="psumb", bufs=2, space="PSUM"))

        # ---- persistent small constants
        ones_col_f = wpool.tile([128, 1], F32, tag="ones_col_f", name="ones_col_f")
        nc.vector.memset(ones_col_f[:], 1.0)
        ones_col_b = wpool.tile([128, 1], BF16, tag="ones_col_b", name="ones_col_b")
        nc.vector.memset(ones_col_b[:], 1.0)
        ones_col_r = wpool.tile([128, 1], F32R, tag="ones_col_r", name="ones_col_r")
        nc.vector.tensor_copy(ones_col_r[:], ones_col_f[:])
        ones1_b = wpool.tile([1, 128], BF16, tag="ones1_b", name="ones1_b")
        nc.vector.memset(ones1_b[:], 1.0)
        dwb_t = wpool.tile([128, DC], F32, tag="dwb", name="dwb")
        nc.sync.dma_start(dwb_t[:], dwb_d)
        p1b_t = wpool.tile([128, MLPC], F32, tag="p1b", name="p1b")
        nc.sync.dma_start(p1b_t[:], p1b_d)
        p2b_t = wpool.tile([128, DC], F32, tag="p2b", name="p2b")
        nc.sync.dma_start(p2b_t[:], p2b_d)
        eps_t = wpool.tile([1, 1], F32, tag="eps", name="eps")
        nc.vector.memset(eps_t[:], EPS)

        # ---- all weights resident in SBUF (bf16, loaded once)
        def load_w(src_ap, shape, tag):
            t = wpool.tile(shape, BF16, tag=tag, name=tag)
            nc.sync.dma_start(t[:], src_ap)
            return t

        whg_w = [[load_w(d[k * 128:(k + 1) * 128, :], [128, 2 * DI],
                         f"whg{g}_{k}")
                  for k in range(DC)] for g, d in enumerate((whg1_d, whg2_d))]
        wout_w = [[load_w(d[k * 128:(k + 1) * 128, :], [128, D],
                          f"wout{g}_{k}")
                   for k in range(DIC)] for g, d in enumerate((wout1_d, wout2_d))]
        p1w = [load_w(p1_d[k * 128:(k + 1) * 128, :], [128, MLP], f"p1_{k}")
               for k in range(DC)]
        p2w = [load_w(p2_d[k * 128:(k + 1) * 128, :], [128, D], f"p2_{k}")
               for k in range(MLPC)]
        dgw = [load_w(diag_d[c], [128, 9 * 128], f"dg{c}") for c in range(DC)]

        ob1 = range(NBLK)
        ob2 = range(NBLK - 1, -1, -1)

        # ---------------- phase helpers (explicit tiles, manual sequencing)
        def alloc_rows():
            """numu at [0:L], inv at [L:2L] -- bf16 rows on partition 0."""
            return work.tile([1, 2 * L], BF16, tag="mmrows", name="mmrows",
                             bufs=2)

        def n1_dma(b, x_t):
            for c in range(DC):
                for blk in ob1:
                    bsl = slice(blk * NT, (blk + 1) * NT)
                    nc.sync.dma_start(x_t[c][:, bsl],
                                      xT_d[b, c * 128:(c + 1) * 128, bsl])

        def norm_stats(rows, src_mm, sum_ones, order):
            """src_mm(c, sl) -> matmul-ready AP (bf16 or f32r)."""
            numu_row = rows[0:1, 0:L]
            inv_row = rows[0:1, L:2 * L]
            strows = work.tile([97, NT], F32, tag="strows", name="strows",
                               bufs=1)
            for blk in order:
                sl = slice(blk * NT, (blk + 1) * NT)
                s_ps = psum.tile([1, NT], F32, tag="pA", name="s_ps")
                q_ps = psum.tile([1, NT], F32, tag="pB", name="q_ps")
                for c in range(DC):
                    sq = work.tile([128, NT], BF16, tag="sq", name="sq",
                                   bufs=1)
                    src = src_mm(c, sl)
                    nc.scalar.activation(sq[:], src, AF.Square)
                    nc.tensor.matmul(s_ps[:], sum_ones[:], src,
                                     start=(c == 0), stop=(c == DC - 1))
                    nc.tensor.matmul(q_ps[:], ones_col_b[:], sq[:],
                                     start=(c == 0), stop=(c == DC - 1))
                nc.scalar.activation(numu_row[:, sl], s_ps[:], AF.Copy,
                                     scale=-1.0 / D)
                m2 = strows[64:65, 0:NT]
                nc.scalar.activation(m2, s_ps[:], AF.Square, scale=1.0 / D)
                ve = strows[96:97, 0:NT]
                nc.vector.scalar_tensor_tensor(ve, q_ps[:], 1.0 / D,
                                               m2, ALU.mult, ALU.subtract)
                sd_blk = strows[32:33, 0:NT]
                nc.scalar.activation(sd_blk, ve, AF.Sqrt, bias=eps_t[:])
                pkb = work.tile([7, NB], F32, tag="pk", name="pkb", bufs=2)
                nc.sync.dma_start(pkb[:], sd_blk)
                ikb = work.tile([7, NB], BF16, tag="ipk", name="ikb",
                                bufs=2)
                with nc.allow_low_precision(reason="bf16 norm rows"):
                    nc.vector.reciprocal(ikb[:], pkb[:])
                nc.sync.dma_start(inv_row[:, sl], ikb[:])

        def norm_apply(rows, dst_tiles, src_any, order):
            """dst = (x + (-mu)) * inv, both rows broadcast via K=1 mms."""
            numu_row = rows[0:1, 0:L]
            inv_row = rows[0:1, L:2 * L]
            for blk in order:
                sl = slice(blk * NT, (blk + 1) * NT)
                mb_ps = psum.tile([128, NT], F32,
                                  tag=("pC" if blk % 2 == 0 else "pA"),
                                  name="mb_ps")
                nc.tensor.matmul(mb_ps[:], ones1_b[:], numu_row[:, sl],
                                 start=True, stop=True)
                ib_ps = psum.tile([128, NT], F32,
                                  tag=("pD" if blk % 2 == 0 else "pB"),
                                  name="ib_ps")
                nc.tensor.matmul(ib_ps[:], ones1_b[:], inv_row[:, sl],
                                 start=True, stop=True)
                for c in range(DC):
                    t = work.tile([128, NT], F32, tag="t_ap", name="t_ap",
                                  bufs=2)
                    nc.vector.tensor_tensor(t[:], src_any(c, sl), mb_ps[:],
                                            ALU.add)
                    nc.vector.tensor_tensor(dst_tiles[c][:, sl], t[:],
                                            ib_ps[:], ALU.mult)

        def conv_phase(xh_t, hc_t):
            for c in range(DC):
                dg = dgw[c]
                for slab in ob1:
                    w0 = max(0, 7 * slab - 1)
                    w1 = min(NB, 7 * slab + 8)
                    nw = w1 - w0
                    win = xh_t[c][:, w0 * NB:w1 * NB]
                    xm = work.tile([128, 512], BF16, tag="xsh0", name="xm",
                                   bufs=1)
                    nc.vector.tensor_copy(xm[:, 1:nw * NB],
                                          win[:, 0:nw * NB - 1])
                    xm3 = xm[:, 0:nw * NB].rearrange("p (r cc) -> p r cc",
                                                     cc=NB)
                    nc.vector.tensor_scalar(xm3[:, :, 0:1], xm3[:, :, 0:1],
                                            0.0, None, ALU.mult)
                    xp = work.tile([128, 512], BF16, tag="xsh1", name="xp",
                                   bufs=1)
                    nc.vector.tensor_copy(xp[:, 0:nw * NB - 1],
                                          win[:, 1:nw * NB])
                    xp3 = xp[:, 0:nw * NB].rearrange("p (r cc) -> p r cc",
                                                     cc=NB)
                    nc.vector.tensor_scalar(xp3[:, :, NB - 1:NB],
                                            xp3[:, :, NB - 1:NB],
                                            0.0, None, ALU.mult)
                    cp = psumb.tile([128, NT], F32,
                                    tag=("hp_ps" if slab % 2 == 0 else "gp_ps"),
                                    name="conv_ps")
                    tap_order = [0, 3, 6, 2, 5, 8, 1, 4, 7]
                    for ti, tap in enumerate(tap_order):
                        dr, dcc, r0, r1, c0, c1 = _conv_tap_ranges(tap, slab)
                        osl = slice((r0 - 7 * slab) * NB, (r1 - 7 * slab) * NB)
                        if dcc == 0:
                            rhs = xh_t[c][:, (r0 + dr) * NB:(r1 + dr) * NB]
                        elif dcc == -1:
                            rhs = xm[:, (r0 + dr - w0) * NB:(r1 + dr - w0) * NB]
                        else:
                            rhs = xp[:, (r0 + dr - w0) * NB:(r1 + dr - w0) * NB]
                        nc.tensor.matmul(
                            cp[:, osl], dg[:, tap * 128:(tap + 1) * 128], rhs,
                            start=(ti == 0), stop=(ti == 8))
                    nc.scalar.activation(
                        hc_t[c][:, slab * NT:(slab + 1) * NT], cp[:],
                        AF.Identity, bias=dwb_t[:, c:c + 1])

        def gru_phase(hc_t, y_t):
            """Bi-minGRU, software-pipelined at both the j level (scan stage
            deferred one j so the DVE queue orders max(j+1) before scan(j))
            and the unit level (out-projections deferred one quarter)."""
            units = [(0, qi, q) for qi, q in enumerate(range(NQ))] +                     [(1, qi, q) for qi, q in enumerate(range(NQ - 1, -1, -1))]

            def unit_out(g, q, hs):
                wout = wout_w[g]
                for dc in range(DC):
                    for nb2 in range(2):
                        y_ps = psum.tile(
                            [128, NT], F32,
                            tag=("pC" if (dc * 2 + nb2) % 2 == 0 else "pD"),
                            name="y_ps")
                        for k in range(DIC):
                            nc.tensor.matmul(
                                y_ps[:],
                                wout[k][:, dc * 128:(dc + 1) * 128],
                                hs[k][:, nb2 * NT:(nb2 + 1) * NT],
                                start=(k == 0), stop=(k == DIC - 1))
                        ysl = slice(q * QT + nb2 * NT,
                                    q * QT + (nb2 + 1) * NT)
                        if g == 0:
                            nc.scalar.activation(y_t[dc][:, ysl], y_ps[:],
                                                 AF.Copy)
                        else:
                            nc.vector.tensor_tensor(
                                y_t[dc][:, ysl],
                                y_t[dc][:, ysl].bitcast(F32), y_ps[:],
                                ALU.add)

            hs_prev = [None]

            def scan_stage(g, qi, j, z, s, hs):
                bb = work.tile([128, QT], F32, tag="bb", name="bb",
                               bufs=3)
                nc.gpsimd.tensor_tensor(bb[:], z[:], s[:], ALU.mult)
                # a = 1 - z in place on z (after bb consumed z)
                nc.gpsimd.tensor_scalar(z[:], z[:], -1.0, 1.0,
                                        ALU.mult, ALU.add)
                if qi == 0:
                    init = 0.0
                elif g == 0:
                    init = hs_prev[0][j][:, QT - 1:QT]
                else:
                    init = hs_prev[0][j][:, 0:1]
                if g == 0:
                    nc.vector.tensor_tensor_scan(
                        hs[j][:], z[:], bb[:], init, ALU.mult, ALU.add)
                else:
                    rv = slice(None, None, -1)
                    nc.vector.tensor_tensor_scan(
                        hs[j][:, rv], z[:, rv], bb[:, rv], init,
                        ALU.mult, ALU.add)

            pend = None          # (g, q, hs) awaiting out-projection
            for g, qi, q in units:
                whg = whg_w[g]
                hs = [work.tile([128, QT], BF16, tag=f"hs{j}",
                                name=f"hs{j}", bufs=2) for j in range(DIC)]
                jps = []         # pending scan stages (depth 2)
                for j in range(DIC):
                    z = work.tile([128, QT], F32, tag="z", name="z", bufs=3)
                    s = work.tile([128, QT], F32, tag="s", name="s", bufs=3)
                    for nb2 in range(2):
                        nsl = slice(q * QT + nb2 * NT,
                                    q * QT + (nb2 + 1) * NT)
                        hsl = slice(nb2 * NT, (nb2 + 1) * NT)
                        hp = psumb.tile([128, NT], F32, tag="hp_ps",
                                        name="hp_ps")
                        gp = psumb.tile([128, NT], F32, tag="gp_ps",
                                        name="gp_ps")
                        for k in range(DC):
                            nc.tensor.matmul(
                                hp[:], whg[k][:, j * 128:(j + 1) * 128],
                                hc_t[k][:, nsl],
                                start=(k == 0), stop=(k == DC - 1))
                        for k in range(DC):
                            nc.tensor.matmul(
                                gp[:],
                                whg[k][:, DI + j * 128:DI + (j + 1) * 128],
                                hc_t[k][:, nsl],
                                start=(k == 0), stop=(k == DC - 1))
                        nc.scalar.activation(z[:, hsl], gp[:], AF.Sigmoid)
                        nc.scalar.activation(s[:, hsl], hp[:], AF.Sigmoid)
                        # g = max(hidden+0.5, sigmoid(hidden)) in place
                        nc.vector.scalar_tensor_tensor(
                            s[:, hsl], hp[:], 0.5, s[:, hsl],
                            ALU.add, ALU.max)
                    jps.append((j, z, s))
                    if len(jps) > 2:
                        scan_stage(g, qi, *jps.pop(0), hs)
                for jp in jps:
                    scan_stage(g, qi, *jp, hs)
                if pend is not None:
                    unit_out(*pend)
                hs_prev[0] = hs
                pend = (g, q, hs)
            unit_out(*pend)

        def resid_phase(b, y_t):
            for blk in ob2:
                for c in range(DC):
                    sl = slice(blk * NT, (blk + 1) * NT)
                    xr = work.tile([128, NT], F32, tag="xr", name="xr",
                                   bufs=2)
                    nc.sync.dma_start(xr[:],
                                      xTf_d[b, c * 128:(c + 1) * 128, sl])
                    nc.vector.tensor_tensor(y_t[c][:, sl],
                                            y_t[c][:, sl].bitcast(F32),
                                            xr[:], ALU.add)

        def mlp_phase(b, yh_t, y_t):
            for blk in ob2:
                sl = slice(blk * NT, (blk + 1) * NT)
                o_ps = []
                for dc in range(DC):
                    if dc < 2:
                        o_ps.append(psum.tile([128, NT], F32,
                                              tag=("pC" if dc == 0 else "pD"),
                                              name=f"o_ps{dc}"))
                    else:
                        o_ps.append(psumb.tile([128, NT], F32, tag="hp_ps",
                                               name="o_ps2"))
                qs = []
                for mc in range(MLPC):
                    q_ps = psum.tile([128, NT], F32,
                                     tag=("pA" if mc % 2 == 0 else "pB"),
                                     name="q_ps_m")
                    for k in range(DC):
                        nc.tensor.matmul(
                            q_ps[:], p1w[k][:, mc * 128:(mc + 1) * 128],
                            yh_t[k][:, sl],
                            start=(k == 0), stop=(k == DC - 1))
                    qt = work.tile([128, NT], BF16, tag=f"hs{mc % 6}",
                                   name=f"q_sb{mc}", bufs=2)
                    nc.scalar.activation(qt[:], q_ps[:], AF.Gelu,
                                         bias=p1b_t[:, mc:mc + 1])
                    qs.append(qt)
                for dc in range(DC):
                    for mc in range(MLPC):
                        nc.tensor.matmul(
                            o_ps[dc][:],
                            p2w[mc][:, dc * 128:(dc + 1) * 128],
                            qs[mc][:],
                            start=(mc == 0), stop=(mc == MLPC - 1))
                for dc in range(DC):
                    oo = work.tile([128, NT], F32, tag="t_ap",
                                   name="oo", bufs=2)
                    nc.vector.scalar_tensor_tensor(
                        oo[:], o_ps[dc][:], p2b_t[:, dc:dc + 1],
                        y_t[dc][:, sl].bitcast(F32), ALU.add, ALU.add)
                    nc.sync.dma_start(
                        out_d[b, dc * 128:(dc + 1) * 128, sl], oo[:])

        def front_half(b):
            """x load -> norm1 stats (PE-light; good phase-gap filler)."""
            x_t = [big.tile([128, L], BF16, tag=f"bufB{c}", name=f"x{c}")
                   for c in range(DC)]
            n1_dma(b, x_t)
            rows = alloc_rows()
            norm_stats(rows, lambda c, sl: x_t[c][:, sl], ones_col_b, ob1)
            return x_t, rows

        def mid_half(b, x_t, rows):
            """norm1 apply -> conv -> GRU."""
            xh_t = [big.tile([128, L], BF16, tag=f"bufC{c}", name=f"xh{c}")
                    for c in range(DC)]
            norm_apply(rows, xh_t, lambda c, sl: x_t[c][:, sl], ob1)
            hc_t = [big.tile([128, L], BF16, tag=f"bufB{c}", name=f"hc{c}")
                    for c in range(DC)]
            conv_phase(xh_t, hc_t)
            y_t = [big.tile([128, L], F32R, tag=f"bufA{c}", name=f"y{c}")
                   for c in range(DC)]
            gru_phase(hc_t, y_t)
            return y_t

        def back_half(b, y_t):
            """residual -> norm2 -> MLP -> out."""
            resid_phase(b, y_t)
            rows = alloc_rows()
            norm_stats(rows, lambda c, sl: y_t[c][:, sl], ones_col_r, ob2)
            yh_t = [big.tile([128, L], BF16, tag=f"bufC{c}", name=f"yh{c}")
                    for c in range(DC)]
            norm_apply(rows, yh_t,
                       lambda c, sl: y_t[c][:, sl].bitcast(F32), ob2)
            mlp_phase(b, yh_t, y_t)

        # ---------------- schedule: b1's PE-light front half is issued
        # between b0's GRU and b0's back half so its stats matmuls fill the
        # residual/norm2 dependency-chain bubble.
        for rep in range(reps):
            x0, rows0 = front_half(0)
            y0 = mid_half(0, x0, rows0)
            x1, rows1 = front_half(1)
            back_half(0, y0)
            y1 = mid_half(1, x1, rows1)
            back_half(1, y1)

    return nc


# ---------------------------------------------------------------- host side
_NC_CACHE = {}


def _get_nc():
    key = "bf16"
    if key not in _NC_CACHE:
        nc = build_kernel()
        _fix_multiwaits(nc)
        _NC_CACHE[key] = nc
    return _NC_CACHE[key]


def _prep_weights(inp):
    f = np.float32
    bf = ml_dtypes.bfloat16
    dw_w = np.asarray(inp["dw_w"], f)          # [D,1,3,3]
    norm_w = np.asarray(inp["norm_w"], f)
    norm_b = np.asarray(inp["norm_b"], f)
    dw_wf = dw_w[:, 0] * norm_w[:, None, None]     # [D,3,3]
    dw_bf = np.asarray(inp["dw_b"], f) + norm_b * dw_w[:, 0].sum(axis=(1, 2))
    p1_w = np.asarray(inp["p1_w"], f)
    p1f = p1_w * np.asarray(inp["norm2_w"], f)[:, None]
    p1bf = np.asarray(inp["p1_b"], f) + np.asarray(inp["norm2_b"], f) @ p1_w

    # conv diagonal weight blocks: [DC, 128, 9*128]
    diag = np.zeros((DC, 128, 9 * 128), f)
    ar = np.arange(128)
    for c in range(DC):
        for tap in range(9):
            dr, dcc = tap // 3, tap % 3
            diag[c, ar, tap * 128 + ar] = dw_wf[c * 128:(c + 1) * 128, dr, dcc]

    return dict(
        whg1=np.ascontiguousarray(np.asarray(inp["gru1_whg"], f)).astype(bf),
        whg2=np.ascontiguousarray(np.asarray(inp["gru2_whg"], f)).astype(bf),
        wout1=np.ascontiguousarray(np.asarray(inp["gru1_wout"], f)).astype(bf),
        wout2=np.ascontiguousarray(np.asarray(inp["gru2_wout"], f)).astype(bf),
        p1=np.ascontiguousarray(p1f).astype(bf),
        p2=np.ascontiguousarray(np.asarray(inp["p2_w"], f)).astype(bf),
        diag=diag.astype(bf),
        dwb=np.ascontiguousarray(dw_bf.reshape(DC, 128).T, f),
        p1b=np.ascontiguousarray(p1bf.reshape(MLPC, 128).T, f),
        p2b=np.ascontiguousarray(np.asarray(inp["p2_b"], f).reshape(DC, 128).T, f),
    )


def _build_in_maps(x, w):
    """x: [16, L, D] f32.  Returns per-core input maps."""
    in_maps = []
    for core in range(NCORES):
        xb = x[core * B:(core + 1) * B]                   # [B, L, D]
        xT = np.ascontiguousarray(xb.transpose(0, 2, 1))  # [B, D, L] f32
        m = dict(w)
        m["xTf"] = xT
        m["xT"] = xT.astype(ml_dtypes.bfloat16)
        in_maps.append(m)
    return in_maps


def kernel(**inputs):
    x = np.asarray(inputs["x"], np.float32)    # [16, L, D]
    w = _prep_weights(inputs)
    nc = _get_nc()
    in_maps = _build_in_maps(x, w)
    res = run_bass_kernel_spmd(nc, in_maps, core_ids=list(range(NCORES)))
    outs = []
    for core in range(NCORES):
        oT = res.results[core]["outT"]                    # [B, D, L]
        outs.append(oT.transpose(0, 2, 1))                # [B, L, D]
    return np.ascontiguousarray(np.concatenate(outs, axis=0), np.float32)
